# revision 12
# baseline (speedup 1.0000x reference)
"""Trainium2 Bass kernel for nn_DocREModel (8-core SPMD).

Sharding: data-parallel over the 4 documents x 2 pair-halves = 8 cores.
Each core runs an identical program; per-core behavior differs only via
its input data (its doc's tensors + its half of the pair one-hots).

All floating-point arithmetic runs on device. Host does only index-driven
data movement: batch slicing, transposes, row gathers at integer indices,
and one-hot/selector matrix construction.
"""

import numpy as np
from contextlib import ExitStack

import concourse.bass as bass
import concourse.bacc as bacc
import concourse.tile as tile
import concourse.mybir as mybir
from concourse.bass_utils import run_bass_kernel_spmd

FP32 = mybir.dt.float32
BF16 = mybir.dt.bfloat16

# compute dtypes per stage
SEQ_DT = BF16    # seq transform matmuls
CONV_DT = BF16   # conv stack
PAIR_DT = BF16   # pair-classification matmuls
GRAPH_DT = BF16  # rgcn / entity-attention matmuls

import ml_dtypes

_NPDT = {FP32: np.float32, BF16: ml_dtypes.bfloat16}

B, C, H, NH = 4, 1024, 768, 12
E, M, L, LS = 22, 3, 30, 16
NN, NF, EMB = 118, 532, 512
P, PH = 462, 231
IC = 256
S = 22            # spatial side of relation map
SP = S * S        # 484
PW = S + 4        # 26 padded side
SPP = PW * PW     # 676
ACT = mybir.ActivationFunctionType
KT_H = H // 128   # 6
ATTM_ROWS = E * M * NH          # 792
ATTM_TILES = [128] * 6 + [24]
SPAN_ROWS = L * LS              # 480
SPAN_TILES = [128, 128, 128, 96]
NF_TILES = [128, 128, 128, 128, 20]   # 532
SP_TILES = [128, 128, 128, 100]       # 484


def _ts(sizes):
    """(offset, size) pairs for a tiling."""
    off = 0
    for sz in sizes:
        yield off, sz
        off += sz


def build_program():
    nc = bacc.Bacc("TRN2", target_bir_lowering=False, debug=False)

    dins = {}

    def din(name, shape, dt=FP32):
        dins[name] = nc.dram_tensor(name, shape, dt, kind="ExternalInput").ap()
        return dins[name]

    xt = din("xt", [H, C], SEQ_DT)            # X.T
    xg = din("xg", [H, E * M], SEQ_DT)        # X rows at mention idx, transposed
    xspan = din("xspan", [SPAN_ROWS, H], SEQ_DT)  # X rows at span positions
    attm = din("attm", [ATTM_ROWS, C], GRAPH_DT)  # attention mention rows, (e,m)*12+h major
    attl = din("attl", [SPAN_ROWS, NH * LS], FP32)  # link blocks, row l*16+j, free (h,i)
    adjt = din("adjt", [NN, 4 * NN], FP32)    # col r*118+i = adjacency[r,i,:]
    typ = din("typ", [NN, 20], FP32)          # type_embed[node_types]
    wtrans = din("wtrans", [H, EMB], SEQ_DT)
    btrans = din("btrans", [1, EMB], FP32)
    gmat = din("gmat", [ATTM_ROWS, E], GRAPH_DT)  # kron(I22, ones(36))/36
    g3 = din("g3", [E * M, E], FP32)          # kron(I22, ones(3))
    gspan = din("gspan", [SPAN_ROWS, L], SEQ_DT)  # kron(I30, ones(16))
    ones = din("ones", [128, 1], FP32)
    ident = din("ident", [128, 128], FP32)
    identp = din("identp", [128, 128], PAIR_DT)
    wrel = din("wrel", [4 * NF, EMB], GRAPH_DT)   # rows r*532+k
    wself = din("wself", [NF, EMB], GRAPH_DT)
    brgcn = din("brgcn", [EMB, 1], FP32)
    w1t = din("w1t", [25 * EMB, IC], CONV_DT)   # rows tap*512+ic
    b1 = din("b1", [IC, 1], FP32)
    w2t = din("w2t", [25 * IC, IC], CONV_DT)
    b2 = din("b2", [IC, 1], FP32)
    w3t = din("w3t", [25 * IC, EMB], CONV_DT)
    b3 = din("b3", [EMB, 1], FP32)
    sh = din("sh", [E, PH], FP32)
    st = din("st", [E, PH], FP32)
    sm = din("sm", [SP, PH], PAIR_DT)
    wht = din("wht", [4 * EMB, 2 * EMB], PAIR_DT)
    bht = din("bht", [2 * EMB, 1], FP32)
    wbil = din("wbil", [2 * EMB, 97], PAIR_DT)
    bbil = din("bbil", [97, 1], FP32)
    outt = nc.dram_tensor("outt", [97, PH], FP32, kind="ExternalOutput").ap()

    with tile.TileContext(nc) as tc, ExitStack() as ctx:
        pp = ctx.enter_context(tc.tile_pool(name="persist", bufs=1))
        pst = ctx.enter_context(tc.tile_pool(name="stream", bufs=1))
        pps = ctx.enter_context(tc.tile_pool(name="psum", bufs=8, space="PSUM"))


        dma = nc.sync.dma_start

        def T(pool, shape, dt, tag, bufs=None):
            return pool.tile(shape, dt, tag=tag, name=tag, bufs=bufs)


        # ---- persistent small tiles ----
        ident_t = T(pp, [128, 128], FP32, "ident")
        dma(ident_t[:], ident)
        identp_t = T(pp, [128, 128], PAIR_DT, "identp")
        dma(identp_t[:], identp)
        ones_t = T(pp, [128, 1], FP32, "ones")
        dma(ones_t[:], ones)
        btrans_t = T(pp, [1, EMB], FP32, "btrans")
        dma(btrans_t[:], btrans)
        btrans_bc = T(pp, [128, EMB], FP32, "btrans_bc")
        nc.gpsimd.partition_broadcast(btrans_bc[:], btrans_t[:])
        nodes_e = T(pp, [E, NF], FP32, "nodes_e")
        nodes_m = T(pp, [E * M, NF], FP32, "nodes_m")
        nodes_l = T(pp, [L, NF], FP32, "nodes_l")
        dma(nodes_e[:, EMB:NF], typ[0:E, :])
        dma(nodes_m[:, EMB:NF], typ[E:E + E * M, :])
        dma(nodes_l[:, EMB:NF], typ[E + E * M:NN, :])

        # ---- S1: seq = X @ W_trans + b  (natural layout [1024 tok, 512]) ----
        wtrans_t = []
        for kt in range(KT_H):
            t = T(pp, [128, EMB], SEQ_DT, f"wtrans{kt}")
            dma(t[:], wtrans[kt * 128:(kt + 1) * 128, :])
            wtrans_t.append(t)

        ps_seq = [T(pps, [128, EMB], FP32, "ps") for _ in range(8)]
        for kt in range(KT_H):
            xt_t = T(pst, [128, C], SEQ_DT, "xt_stream", bufs=2)
            dma(xt_t[:], xt[kt * 128:(kt + 1) * 128, :])
            for mt in range(8):
                nc.tensor.matmul(
                    ps_seq[mt][:], xt_t[:, mt * 128:(mt + 1) * 128], wtrans_t[kt][:],
                    start=(kt == 0), stop=(kt == KT_H - 1))
        seq_t = []
        for mt in range(8):
            t = T(pp, [128, EMB], SEQ_DT, f"seq{mt}")
            nc.vector.tensor_add(t[:], ps_seq[mt][:], btrans_bc[:])
            seq_t.append(t)

        # ---- S2: mention embeddings + entity logsumexp nodes ----
        ps_memb = T(pps, [E * M, EMB], FP32, "ps")
        for kt in range(KT_H):
            xg_t = T(pst, [128, E * M], SEQ_DT, "xg_stream", bufs=3)
            dma(xg_t[:], xg[kt * 128:(kt + 1) * 128, :])
            nc.tensor.matmul(ps_memb[:], xg_t[:], wtrans_t[kt][:],
                             start=(kt == 0), stop=(kt == KT_H - 1))
        memb_t = T(pp, [E * M, EMB], FP32, "memb")
        nc.vector.tensor_add(memb_t[:], ps_memb[:], btrans_bc[0:E * M, :])
        nc.vector.tensor_copy(nodes_m[:, 0:EMB], memb_t[:])
        ememb_t = T(pp, [E * M, EMB], FP32, "ememb")
        nc.scalar.activation(ememb_t[:], memb_t[:], ACT.Exp)
        g3_t = T(pp, [E * M, E], FP32, "g3")
        dma(g3_t[:], g3)
        ps_ent = T(pps, [E, EMB], FP32, "ps")
        nc.tensor.matmul(ps_ent[:], g3_t[:], ememb_t[:], start=True, stop=True)
        nc.scalar.activation(nodes_e[:, 0:EMB], ps_ent[:], ACT.Ln)

        # ---- S3: link nodes ----
        # a[s] = mean over (h,i) of the 16x16 link attention block, s=(l,j)
        aT_t, aTb_t, xspan_t, gspan_t = [], [], [], []
        for i, (off, sz) in enumerate(_ts(SPAN_TILES)):
            al = T(pst, [sz, NH * LS], FP32, "attl_stream", bufs=2)
            dma(al[:], attl[off:off + sz, :])
            a = T(pp, [sz, 1], FP32, f"aT{i}")
            nc.vector.tensor_reduce(a[:], al[:], mybir.AxisListType.X,
                                    mybir.AluOpType.add)
            nc.vector.tensor_scalar_mul(a[:], a[:], 1.0 / (NH * LS))
            aT_t.append(a)
            ab = T(pp, [sz, 1], SEQ_DT, f"aTb{i}")
            nc.vector.tensor_copy(ab[:], a[:])
            aTb_t.append(ab)
            gs = T(pp, [sz, L], SEQ_DT, f"gspan{i}")
            dma(gs[:], gspan[off:off + sz, :])
            gspan_t.append(gs)
            xs = T(pp, [sz, H], SEQ_DT, f"xspan{i}")
            dma(xs[:], xspan[off:off + sz, :])
            xspan_t.append(xs)
        # asum[l] = sum_j a_l[j] (for the bias term); uses unscaled-by-X a
        ps_as = T(pps, [L, 1], FP32, "ps")
        for kt in range(4):
            nc.tensor.matmul(ps_as[:], gspan_t[kt][:], aTb_t[kt][:],
                             start=(kt == 0), stop=(kt == 3))
        asum_t = T(pp, [L, 1], FP32, "asum")
        nc.vector.tensor_copy(asum_t[:], ps_as[:])
        # scale xspan rows by a in place, then project through gspan
        for kt in range(4):
            nc.vector.tensor_scalar_mul(xspan_t[kt][:], xspan_t[kt][:],
                                        aT_t[kt][:])
        # linkctxT [768, 30]
        lct_t = []
        for mt in range(KT_H):
            ps = T(pps, [128, L], FP32, "ps")
            for kt in range(4):
                nc.tensor.matmul(ps[:], xspan_t[kt][:, mt * 128:(mt + 1) * 128],
                                 gspan_t[kt][:], start=(kt == 0), stop=(kt == 3))
            t = T(pp, [128, L], SEQ_DT, f"lct{mt}")
            nc.vector.tensor_copy(t[:], ps[:])
            lct_t.append(t)
        bterm_t = T(pp, [L, EMB], FP32, "bterm")
        nc.vector.tensor_scalar_mul(bterm_t[:], btrans_bc[0:L, :], asum_t[:])
        ps_link = T(pps, [L, EMB], FP32, "ps")
        for kt in range(KT_H):
            nc.tensor.matmul(ps_link[:], lct_t[kt][:], wtrans_t[kt][:],
                             start=(kt == 0), stop=(kt == KT_H - 1))
        nc.vector.tensor_add(nodes_l[:, 0:EMB], ps_link[:], bterm_t[:])

        # ---- S4: ea (entity attention) + e_ctx ----
        ps_ea = [T(pps, [E, 512], FP32, "ps") for _ in range(2)]
        n_attm = len(ATTM_TILES)
        for i, (off, sz) in enumerate(_ts(ATTM_TILES)):
            at = T(pst, [sz, C], GRAPH_DT, "attm_stream", bufs=2)
            dma(at[:], attm[off:off + sz, :])
            gt = T(pst, [sz, E], GRAPH_DT, "gmat_stream", bufs=3)
            dma(gt[:], gmat[off:off + sz, :])
            for half in range(2):
                nc.tensor.matmul(ps_ea[half][:], gt[:],
                                 at[:, half * 512:(half + 1) * 512],
                                 start=(i == 0), stop=(i == n_attm - 1))
        ea_t = T(pp, [E, C], FP32, "ea")
        for half in range(2):
            nc.vector.tensor_copy(ea_t[:, half * 512:(half + 1) * 512],
                                  ps_ea[half][:])
        rsum_t = T(pp, [E, 1], FP32, "rsum")
        nc.vector.tensor_reduce(rsum_t[:], ea_t[:], mybir.AxisListType.X,
                                mybir.AluOpType.add)
        nc.vector.tensor_scalar_add(rsum_t[:], rsum_t[:], 1e-5)
        recip_t = T(pp, [E, 1], FP32, "recip")
        nc.vector.reciprocal(recip_t[:], rsum_t[:])
        ean_t = T(pp, [E, C], FP32, "ean")
        nc.vector.tensor_scalar_mul(ean_t[:], ea_t[:], recip_t[:])
        # eaNT via PE transpose, then e_ctx [22, 512]
        ps_ectx = T(pps, [E, EMB], FP32, "ps")
        for kt in range(8):
            pst_ea = T(pps, [128, E], FP32, "ps")
            nc.tensor.transpose(pst_ea[:], ean_t[:, kt * 128:(kt + 1) * 128],
                                ident_t[0:E, 0:E])
            eaT = T(pst, [128, E], SEQ_DT, "eaT_stream", bufs=3)
            nc.vector.tensor_copy(eaT[:], pst_ea[:])
            nc.tensor.matmul(ps_ectx[:], eaT[:], seq_t[kt][:],
                             start=(kt == 0), stop=(kt == 7))
        ectx_t = T(pp, [E, EMB], FP32, "ectx")
        nc.vector.tensor_copy(ectx_t[:], ps_ectx[:])

        def conv_pass(ps_c, in_tiles, w_dram, n_ic_t, n_oc_t, ocs,
                      first, last):
            """One accumulation pass of a 5x5 SAME conv on 22x22."""
            n_acc = 25 * n_ic_t
            a = 0
            for tap in range(25):
                di, dj = divmod(tap, 5)
                for kt in range(n_ic_t):
                    w = T(pst, [128, ocs], CONV_DT, "wconv_stream", bufs=6)
                    dma(w[:], w_dram[(tap * n_ic_t + kt) * 128:
                                     (tap * n_ic_t + kt + 1) * 128, :])
                    rhs = in_tiles[kt][:].rearrange(
                        "p (a b) -> p a b", a=PW, b=PW)[:, di:di + S, dj:dj + S]
                    for mt in range(n_oc_t):
                        nc.tensor.matmul(ps_c[mt][:], w[:, mt * 128:(mt + 1) * 128],
                                         rhs, start=(first and a == 0),
                                         stop=(last and a == n_acc - 1))
                    a += 1

        def conv(in_tiles, w_dram, n_ic_t, n_oc_t, ocs, bias_tiles, out_cb):
            ps_c = [T(pps, [128, SP], FP32, "ps") for _ in range(n_oc_t)]
            conv_pass(ps_c, in_tiles, w_dram, n_ic_t, n_oc_t, ocs, True, True)
            for mt in range(n_oc_t):
                out_cb(mt, ps_c[mt])

        # ectxT tiles [128, 22] x4
        ectxT_t = []
        for mt in range(4):
            ps = T(pps, [128, E], FP32, "ps")
            nc.tensor.transpose(ps[:], ectx_t[:, mt * 128:(mt + 1) * 128],
                                ident_t[0:E, 0:E])
            t = T(pp, [128, E], FP32, f"ectxT{mt}")
            nc.vector.tensor_copy(t[:], ps[:])
            ectxT_t.append(t)

        # conv1 is linear in x = ent-outer + ectx-outer; run the ectx part
        # now so it overlaps the rgcn dependency chain
        xpad_ec_t = []
        for mt in range(4):
            xp = T(pp, [128, SPP], CONV_DT, f"xpadec{mt}")
            nc.vector.memset(xp[:], 0.0)
            inner = xp[:].rearrange("p (a b) -> p a b", a=PW, b=PW)[:, 2:2 + S, 2:2 + S]
            nc.vector.tensor_mul(
                inner,
                ectxT_t[mt][:].unsqueeze(2).to_broadcast((128, S, S)),
                ectxT_t[mt][:].unsqueeze(1).to_broadcast((128, S, S)))
            xpad_ec_t.append(xp)
        ps_c1 = [T(pps, [128, SP], FP32, "ps") for _ in range(2)]
        conv_pass(ps_c1, xpad_ec_t, w1t, 4, 2, IC, True, False)


        # ---- S5: RGCN ----
        # nodesT [532, 118] built from the three node-group tiles
        NODE_GROUPS = [(0, E), (E, E * M), (E + E * M, L)]
        node_tiles = [nodes_e, nodes_m, nodes_l]
        # bf16 copies of the node features for the rgcn matmuls
        nodes_bf = []
        for gi, (goff, gsz) in enumerate(NODE_GROUPS):
            t = T(pp, [gsz, NF], GRAPH_DT, f"nodes_bf{gi}")
            nc.vector.tensor_copy(t[:], node_tiles[gi][:])
            nodes_bf.append(t)
        nodesT_t = []
        for i, (off, sz) in enumerate(_ts(NF_TILES)):
            t = T(pp, [sz, NN], GRAPH_DT, f"nodesT{i}")
            for gi, (goff, gsz) in enumerate(NODE_GROUPS):
                ps = T(pps, [sz, 128], FP32, "ps")
                nc.tensor.transpose(ps[0:sz, 0:gsz],
                                    node_tiles[gi][:, off:off + sz],
                                    ident_t[0:gsz, 0:gsz])
                nc.vector.tensor_copy(t[:, goff:goff + gsz], ps[0:sz, 0:gsz])
            nodesT_t.append(t)
        # adjacency normalize; adjn split into the three node row-groups
        adjt_t = T(pp, [NN, 4 * NN], FP32, "adjt")
        dma(adjt_t[:], adjt)
        ps_rs = T(pps, [1, 4 * NN], FP32, "ps")
        nc.tensor.matmul(ps_rs[:], ones_t[0:NN, 0:1], adjt_t[:],
                         start=True, stop=True)
        rs_t = T(pp, [1, 4 * NN], FP32, "rs")
        nc.vector.tensor_scalar_add(rs_t[:], ps_rs[:], 1e-5)
        rcp_t = T(pp, [1, 4 * NN], FP32, "rcp")
        nc.vector.reciprocal(rcp_t[:], rs_t[:])
        rsbc_t = T(pp, [128, 4 * NN], FP32, "rsbc")
        nc.gpsimd.partition_broadcast(rsbc_t[:], rcp_t[:])
        adjn_t = []
        for gi, (goff, gsz) in enumerate(NODE_GROUPS):
            tf = T(pst, [gsz, 4 * NN], FP32, "adjn_f32", bufs=3)
            dma(tf[:], adjt[goff:goff + gsz, :])
            t = T(pp, [gsz, 4 * NN], GRAPH_DT, f"adjn{gi}")
            nc.vector.tensor_mul(t[:], tf[:], rsbc_t[0:gsz, :])
            adjn_t.append(t)

        ps_gcn = [T(pps, [128, NN], FP32, "ps") for _ in range(4)]
        n_terms = 5 * 5  # (4 rel + self) x 5 k-tiles
        term = 0
        for r in range(5):
            # r<4: relation r via msgT; r==4: self term via nodesT
            if r < 4:
                msgT_t = []
                for i, (off, sz) in enumerate(_ts(NF_TILES)):
                    ps = T(pps, [sz, NN], FP32, "ps")
                    for gi, (goff, gsz) in enumerate(NODE_GROUPS):
                        nc.tensor.matmul(ps[:], nodes_bf[gi][:, off:off + sz],
                                         adjn_t[gi][:, r * NN:(r + 1) * NN],
                                         start=(gi == 0), stop=(gi == 2))
                    t = T(pst, [sz, NN], GRAPH_DT, f"msgT_stream{i}", bufs=2)
                    nc.vector.tensor_copy(t[:], ps[:])
                    msgT_t.append(t)
                rhs_t = msgT_t
            else:
                rhs_t = nodesT_t
            for i, (off, sz) in enumerate(_ts(NF_TILES)):
                w = T(pst, [sz, EMB], GRAPH_DT, "wg_stream", bufs=3)
                if r < 4:
                    dma(w[:], wrel[r * NF + off:r * NF + off + sz, :])
                else:
                    dma(w[:], wself[off:off + sz, :])
                for mt in range(4):
                    nc.tensor.matmul(ps_gcn[mt][:], w[:, mt * 128:(mt + 1) * 128],
                                     rhs_t[i][:], start=(term == 0),
                                     stop=(term == n_terms - 1))
                term += 1
        # [512,1] needs 4 partition tiles
        brgcn_tiles = []
        for mt in range(4):
            t = T(pp, [128, 1], FP32, f"brgcn{mt}")
            dma(t[:], brgcn[mt * 128:(mt + 1) * 128, :])
            brgcn_tiles.append(t)
        gcnT_t = []
        for mt in range(4):
            t = T(pp, [128, NN], FP32, f"gcnT{mt}")
            nc.scalar.activation(t[:], ps_gcn[mt][:], ACT.Relu,
                                 bias=brgcn_tiles[mt][:, 0:1])
            gcnT_t.append(t)
        # ent natural [22, 512]; entT view = gcnT[:, 0:22]
        ent_t = T(pp, [E, EMB], FP32, "ent")
        for mt in range(4):
            ps = T(pps, [E, 128], FP32, "ps")
            nc.tensor.transpose(ps[:], gcnT_t[mt][:, 0:E], ident_t[:, :])
            nc.vector.tensor_copy(ent_t[:, mt * 128:(mt + 1) * 128], ps[:])
        # ---- S6: relation map x + conv stack ----
        xpad_t = []
        for mt in range(4):
            xp = T(pp, [128, SPP], CONV_DT, f"xpad{mt}")
            nc.vector.memset(xp[:], 0.0)
            entT_v = gcnT_t[mt][:, 0:E]
            inner = xp[:].rearrange("p (a b) -> p a b", a=PW, b=PW)[:, 2:2 + S, 2:2 + S]
            nc.vector.tensor_mul(
                inner,
                entT_v.unsqueeze(2).to_broadcast((128, S, S)),
                entT_v.unsqueeze(1).to_broadcast((128, S, S)))
            xpad_t.append(xp)

        # conv1: 512 -> 256, output into padded tiles for conv2
        pad1_t = []
        for mt in range(2):
            t = T(pp, [128, SPP], CONV_DT, f"pad1_{mt}")
            nc.vector.memset(t[:], 0.0)
            pad1_t.append(t)
        b1_tiles = []
        for mt in range(2):
            t = T(pp, [128, 1], FP32, f"b1_{mt}")
            dma(t[:], b1[mt * 128:(mt + 1) * 128, :])
            b1_tiles.append(t)

        def c1_out(mt, ps):
            inner = pad1_t[mt][:].rearrange("p (a b) -> p a b", a=PW, b=PW)[
                :, 2:2 + S, 2:2 + S]
            nc.scalar.activation(inner, ps[:].rearrange("p (a b) -> p a b", a=S, b=S),
                                 ACT.Relu, bias=b1_tiles[mt][:, 0:1])

        conv_pass(ps_c1, xpad_t, w1t, 4, 2, IC, False, True)
        for mt in range(2):
            c1_out(mt, ps_c1[mt])

        pad2_t = []
        for mt in range(2):
            t = T(pp, [128, SPP], CONV_DT, f"pad2_{mt}")
            nc.vector.memset(t[:], 0.0)
            pad2_t.append(t)
        b2_tiles = []
        for mt in range(2):
            t = T(pp, [128, 1], FP32, f"b2_{mt}")
            dma(t[:], b2[mt * 128:(mt + 1) * 128, :])
            b2_tiles.append(t)

        def c2_out(mt, ps):
            inner = pad2_t[mt][:].rearrange("p (a b) -> p a b", a=PW, b=PW)[
                :, 2:2 + S, 2:2 + S]
            nc.scalar.activation(inner, ps[:].rearrange("p (a b) -> p a b", a=S, b=S),
                                 ACT.Relu, bias=b2_tiles[mt][:, 0:1])

        conv(pad1_t, w2t, 2, 2, IC, b2_tiles, c2_out)

        x3_t = []
        b3_tiles = []
        for mt in range(4):
            t = T(pp, [128, 1], FP32, f"b3_{mt}")
            dma(t[:], b3[mt * 128:(mt + 1) * 128, :])
            b3_tiles.append(t)
        for mt in range(4):
            t = T(pp, [128, SP], PAIR_DT, f"x3_{mt}")
            x3_t.append(t)

        def c3_out(mt, ps):
            nc.scalar.activation(x3_t[mt][:], ps[:], ACT.Relu,
                                 bias=b3_tiles[mt][:, 0:1])

        conv(pad2_t, w3t, 2, 4, EMB, b3_tiles, c3_out)

        # ---- S7: pair features + classifier ----
        # x3T [484, 512]
        x3T_t = []
        for i, (off, sz) in enumerate(_ts(SP_TILES)):
            t = T(pp, [sz, EMB], PAIR_DT, f"x3T{i}")
            x3T_t.append(t)
        for i, (off, sz) in enumerate(_ts(SP_TILES)):
            for src in range(4):
                ps = T(pps, [sz, 64], FP32, "ps")
                psb = ps[:].bitcast(PAIR_DT)
                nc.tensor.transpose(psb, x3_t[src][:, off:off + sz],
                                    identp_t[:, :])
                nc.vector.tensor_copy(x3T_t[i][:, src * 128:(src + 1) * 128], psb)

        sh_t = T(pp, [E, PH], FP32, "sh")
        dma(sh_t[:], sh)
        st_t = T(pp, [E, PH], FP32, "st")
        dma(st_t[:], st)
        sm_t = []
        for i, (off, sz) in enumerate(_ts(SP_TILES)):
            t = T(pp, [sz, PH], PAIR_DT, f"sm{i}")
            dma(t[:], sm[off:off + sz, :])
            sm_t.append(t)

        featT = [None] * 16
        for mt in range(4):
            ps = T(pps, [128, PH], FP32, "ps")
            nc.tensor.matmul(ps[:], ent_t[:, mt * 128:(mt + 1) * 128], sh_t[:],
                             start=True, stop=True)
            t = T(pp, [128, PH], PAIR_DT, f"featT{mt}")
            nc.vector.tensor_copy(t[:], ps[:])
            featT[mt] = t
        for mt in range(4):
            ps = T(pps, [128, PH], FP32, "ps")
            nc.tensor.matmul(ps[:], ent_t[:, mt * 128:(mt + 1) * 128], st_t[:],
                             start=True, stop=True)
            t = T(pp, [128, PH], PAIR_DT, f"featT{4 + mt}")
            nc.vector.tensor_copy(t[:], ps[:])
            featT[4 + mt] = t
        for mt in range(4):
            ps = T(pps, [128, PH], FP32, "ps")
            for i, (off, sz) in enumerate(_ts(SP_TILES)):
                nc.tensor.matmul(ps[:], x3T_t[i][:, mt * 128:(mt + 1) * 128],
                                 sm_t[i][:], start=(i == 0), stop=(i == 3))
            t = T(pp, [128, PH], PAIR_DT, f"featT{8 + mt}")
            nc.vector.tensor_copy(t[:], ps[:])
            featT[8 + mt] = t
        for mt in range(4):
            t = T(pp, [128, PH], PAIR_DT, f"featT{12 + mt}")
            nc.vector.tensor_mul(t[:], featT[mt][:], featT[4 + mt][:])
            featT[12 + mt] = t

        bht_tiles = []
        for mt in range(8):
            t = T(pp, [128, 1], FP32, f"bht{mt}")
            dma(t[:], bht[mt * 128:(mt + 1) * 128, :])
            bht_tiles.append(t)
        ps_ht = [T(pps, [128, PH], FP32, "ps") for _ in range(8)]
        for kt in range(16):
            w = T(pst, [128, 2 * EMB], PAIR_DT, "wht_stream", bufs=2)
            dma(w[:], wht[kt * 128:(kt + 1) * 128, :])
            for mt in range(8):
                nc.tensor.matmul(ps_ht[mt][:], w[:, mt * 128:(mt + 1) * 128],
                                 featT[kt][:], start=(kt == 0), stop=(kt == 15))
        htT_t = []
        for mt in range(8):
            t = T(pp, [128, PH], PAIR_DT, f"htT{mt}")
            nc.scalar.activation(t[:], ps_ht[mt][:], ACT.Tanh,
                                 bias=bht_tiles[mt][:, 0:1])
            htT_t.append(t)

        ps_out = T(pps, [97, PH], FP32, "ps")
        for kt in range(8):
            w = T(pst, [128, 97], PAIR_DT, "wbil_stream", bufs=3)
            dma(w[:], wbil[kt * 128:(kt + 1) * 128, :])
            nc.tensor.matmul(ps_out[:], w[:], htT_t[kt][:],
                             start=(kt == 0), stop=(kt == 7))
        bbil_t = T(pp, [97, 1], FP32, "bbil")
        dma(bbil_t[:], bbil)
        out_t = T(pp, [97, PH], FP32, "out")
        nc.vector.tensor_scalar_add(out_t[:], ps_out[:], bbil_t[:, 0:1])
        dma(outt, out_t[:])

    nc.compile()
    return nc


_PROG = None


def _get_prog():
    global _PROG
    if _PROG is None:
        _PROG = build_program()
    return _PROG


def _np(dt):
    return _NPDT[dt]


def _shared_inputs(inputs):
    f32 = np.float32
    sh = {}
    sh["wtrans"] = np.ascontiguousarray(inputs["W_trans"], _np(SEQ_DT))
    sh["btrans"] = np.ascontiguousarray(inputs["b_trans"], f32).reshape(1, EMB)
    sh["gmat"] = np.kron(np.eye(E, dtype=f32),
                         np.ones((M * NH, 1), f32) / (M * NH)).astype(_np(GRAPH_DT))
    sh["g3"] = np.kron(np.eye(E, dtype=f32), np.ones((M, 1), f32))
    sh["gspan"] = np.kron(np.eye(L, dtype=f32), np.ones((LS, 1), f32)).astype(_np(SEQ_DT))
    sh["ones"] = np.ones((128, 1), f32)
    sh["ident"] = np.eye(128, dtype=f32)
    sh["identp"] = np.eye(128, dtype=_np(PAIR_DT))
    sh["wrel"] = np.ascontiguousarray(inputs["W_rel"], f32).reshape(4 * NF, EMB).astype(_np(GRAPH_DT))
    sh["wself"] = np.ascontiguousarray(inputs["W_self"], f32).astype(_np(GRAPH_DT))
    sh["brgcn"] = np.ascontiguousarray(inputs["b_rgcn"], f32).reshape(EMB, 1)
    sh["w1t"] = np.ascontiguousarray(
        np.asarray(inputs["conv1_w"], f32).transpose(2, 3, 1, 0).reshape(25 * EMB, IC),
        _np(CONV_DT))
    sh["b1"] = np.ascontiguousarray(inputs["conv1_b"], f32).reshape(IC, 1)
    sh["w2t"] = np.ascontiguousarray(
        np.asarray(inputs["conv2_w"], f32).transpose(2, 3, 1, 0).reshape(25 * IC, IC),
        _np(CONV_DT))
    sh["b2"] = np.ascontiguousarray(inputs["conv2_b"], f32).reshape(IC, 1)
    sh["w3t"] = np.ascontiguousarray(
        np.asarray(inputs["conv3_w"], f32).transpose(2, 3, 1, 0).reshape(25 * IC, EMB),
        _np(CONV_DT))
    sh["b3"] = np.ascontiguousarray(inputs["conv3_b"], f32).reshape(EMB, 1)
    sh["wht"] = np.ascontiguousarray(inputs["ht_W"], _np(PAIR_DT))
    sh["bht"] = np.ascontiguousarray(inputs["ht_b"], f32).reshape(2 * EMB, 1)
    sh["wbil"] = np.ascontiguousarray(inputs["bil_W"], _np(PAIR_DT))
    sh["bbil"] = np.ascontiguousarray(inputs["bil_b"], f32).reshape(97, 1)
    return sh


def _core_inputs(inputs, shared, b, hh):
    f32 = np.float32
    X = np.asarray(inputs["sequence_output"][b], f32)
    att = np.asarray(inputs["attention"][b], f32)
    adj = np.asarray(inputs["adjacency"][b], f32)
    mf = np.asarray(inputs["mention_idx"][b]).reshape(-1).astype(np.int64)
    ls = np.asarray(inputs["link_start"][b]).reshape(-1).astype(np.int64)
    ntypes = np.asarray(inputs["node_types"][b]).astype(np.int64)
    hts = np.asarray(inputs["hts"][b]).astype(np.int64)

    m = dict(shared)
    m["xt"] = np.ascontiguousarray(X.T, _np(SEQ_DT))
    m["xg"] = np.ascontiguousarray(X[mf].T, _np(SEQ_DT))
    pos = ls[:, None] + np.arange(LS)
    m["xspan"] = np.ascontiguousarray(X[pos.reshape(-1)]).astype(_np(SEQ_DT))
    rows = att[:, mf, :]
    m["attm"] = np.ascontiguousarray(
        rows.transpose(1, 0, 2).reshape(ATTM_ROWS, C)).astype(_np(GRAPH_DT))
    attl = np.empty((SPAN_ROWS, NH * LS), f32)
    for l in range(L):
        blk = att[:, pos[l], :][:, :, pos[l]]           # [12, 16i, 16j]
        attl[l * LS:(l + 1) * LS, :] = blk.transpose(2, 0, 1).reshape(LS, NH * LS)
    m["attl"] = attl
    m["adjt"] = np.ascontiguousarray(
        np.concatenate([adj[r].T for r in range(4)], axis=1), f32)
    m["typ"] = np.ascontiguousarray(
        np.asarray(inputs["type_embed"], f32)[ntypes], f32)
    pr = hts[hh * PH:(hh + 1) * PH]
    shm = np.zeros((E, PH), f32)
    shm[pr[:, 0], np.arange(PH)] = 1.0
    stm = np.zeros((E, PH), f32)
    stm[pr[:, 1], np.arange(PH)] = 1.0
    smm = np.zeros((SP, PH), f32)
    smm[pr[:, 0] * E + pr[:, 1], np.arange(PH)] = 1.0
    m["sh"] = shm
    m["st"] = stm
    m["sm"] = np.ascontiguousarray(smm, _np(PAIR_DT))
    return m


def kernel(**inputs):
    nc = _get_prog()
    shared = _shared_inputs(inputs)
    in_maps = []
    for b in range(B):
        for hh in range(2):
            in_maps.append(_core_inputs(inputs, shared, b, hh))
    res = run_bass_kernel_spmd(nc, in_maps, core_ids=list(range(8)))
    out = np.empty((B, P, 97), np.float32)
    for b in range(B):
        for hh in range(2):
            out[b, hh * PH:(hh + 1) * PH, :] = np.asarray(
                res.results[2 * b + hh]["outt"], np.float32).T
    return out


# revision 13
# speedup vs baseline: 1.2111x; 1.2111x over previous
"""Trainium2 Bass kernel for nn_DocREModel (8-core SPMD).

Sharding: data-parallel over the 4 documents x 2 pair-halves = 8 cores.
Each core runs an identical program; per-core behavior differs only via
its input data (its doc's tensors + its half of the pair one-hots).

All floating-point arithmetic runs on device. Host does only index-driven
data movement: batch slicing, transposes, row gathers at integer indices,
and one-hot/selector matrix construction.
"""

import numpy as np
from contextlib import ExitStack

import concourse.bass as bass
import concourse.bacc as bacc
import concourse.tile as tile
import concourse.mybir as mybir
from concourse.bass_utils import run_bass_kernel_spmd

FP32 = mybir.dt.float32
BF16 = mybir.dt.bfloat16

# compute dtypes per stage
SEQ_DT = BF16    # seq transform matmuls
CONV_DT = BF16   # conv stack
PAIR_DT = BF16   # pair-classification matmuls
GRAPH_DT = BF16  # rgcn / entity-attention matmuls

import ml_dtypes

_NPDT = {FP32: np.float32, BF16: ml_dtypes.bfloat16}

B, C, H, NH = 4, 1024, 768, 12
E, M, L, LS = 22, 3, 30, 16
NN, NF, EMB = 118, 532, 512
P, PH = 462, 231
IC = 256
S = 22            # spatial side of relation map
SP = S * S        # 484
PW = S + 4        # 26 padded side
SPP = PW * PW     # 676
ACT = mybir.ActivationFunctionType
KT_H = H // 128   # 6
ATTM_ROWS = E * M * NH          # 792
ATTM_TILES = [128] * 6 + [24]
SPAN_ROWS = L * LS              # 480
SPAN_TILES = [128, 128, 128, 96]
NF_TILES = [128, 128, 128, 128, 20]   # 532
SP_TILES = [128, 128, 128, 100]       # 484


def _ts(sizes):
    """(offset, size) pairs for a tiling."""
    off = 0
    for sz in sizes:
        yield off, sz
        off += sz


def build_program():
    nc = bacc.Bacc("TRN2", target_bir_lowering=False, debug=False)

    dins = {}

    def din(name, shape, dt=FP32):
        dins[name] = nc.dram_tensor(name, shape, dt, kind="ExternalInput").ap()
        return dins[name]

    xt = din("xt", [H, C], SEQ_DT)            # X.T
    xg = din("xg", [H, E * M], SEQ_DT)        # X rows at mention idx, transposed
    xspan = din("xspan", [SPAN_ROWS, H], SEQ_DT)  # X rows at span positions
    attm = din("attm", [ATTM_ROWS, C], GRAPH_DT)  # attention mention rows, (e,m)*12+h major
    attl = din("attl", [SPAN_ROWS, NH * LS], FP32)  # link blocks, row l*16+j, free (h,i)
    adjt = din("adjt", [NN, 4 * NN], FP32)    # col r*118+i = adjacency[r,i,:]
    typ = din("typ", [NN, 20], FP32)          # type_embed[node_types]
    wtrans = din("wtrans", [H, EMB], SEQ_DT)
    btrans = din("btrans", [1, EMB], FP32)
    gmat = din("gmat", [ATTM_ROWS, E], GRAPH_DT)  # kron(I22, ones(36))/36
    g3 = din("g3", [E * M, E], FP32)          # kron(I22, ones(3))
    gspan = din("gspan", [SPAN_ROWS, L], SEQ_DT)  # kron(I30, ones(16))
    ones = din("ones", [128, 1], FP32)
    ident = din("ident", [128, 128], FP32)
    identp = din("identp", [128, 128], PAIR_DT)
    wrel = din("wrel", [4 * NF, EMB], GRAPH_DT)   # rows r*532+k
    wself = din("wself", [NF, EMB], GRAPH_DT)
    brgcn = din("brgcn", [EMB, 1], FP32)
    w1t = din("w1t", [25 * EMB, IC], CONV_DT)   # rows tap*512+ic
    b1 = din("b1", [IC, 1], FP32)
    w2t = din("w2t", [25 * IC, IC], CONV_DT)
    b2 = din("b2", [IC, 1], FP32)
    w3t = din("w3t", [25 * IC, EMB], CONV_DT)
    b3 = din("b3", [EMB, 1], FP32)
    sh = din("sh", [E, PH], FP32)
    st = din("st", [E, PH], FP32)
    sm = din("sm", [SP, PH], PAIR_DT)
    wht = din("wht", [4 * EMB, 2 * EMB], PAIR_DT)
    bht = din("bht", [2 * EMB, 1], FP32)
    wbil = din("wbil", [2 * EMB, 97], PAIR_DT)
    bbil = din("bbil", [97, 1], FP32)
    outt = nc.dram_tensor("outt", [97, PH], FP32, kind="ExternalOutput").ap()

    with tile.TileContext(nc) as tc, ExitStack() as ctx:
        pp = ctx.enter_context(tc.tile_pool(name="persist", bufs=1))
        pst = ctx.enter_context(tc.tile_pool(name="stream", bufs=1))
        pps = ctx.enter_context(tc.tile_pool(name="psum", bufs=8, space="PSUM"))


        dma = nc.sync.dma_start

        def T(pool, shape, dt, tag, bufs=None):
            return pool.tile(shape, dt, tag=tag, name=tag, bufs=bufs)


        # ---- persistent small tiles ----
        ident_t = T(pp, [128, 128], FP32, "ident")
        dma(ident_t[:], ident)
        identp_t = T(pp, [128, 128], PAIR_DT, "identp")
        dma(identp_t[:], identp)
        ones_t = T(pp, [128, 1], FP32, "ones")
        dma(ones_t[:], ones)
        btrans_t = T(pp, [1, EMB], FP32, "btrans")
        dma(btrans_t[:], btrans)
        btrans_bc = T(pp, [128, EMB], FP32, "btrans_bc")
        nc.gpsimd.partition_broadcast(btrans_bc[:], btrans_t[:])
        nodes_e = T(pp, [E, NF], FP32, "nodes_e")
        nodes_m = T(pp, [E * M, NF], FP32, "nodes_m")
        nodes_l = T(pp, [L, NF], FP32, "nodes_l")
        dma(nodes_e[:, EMB:NF], typ[0:E, :])
        dma(nodes_m[:, EMB:NF], typ[E:E + E * M, :])
        dma(nodes_l[:, EMB:NF], typ[E + E * M:NN, :])

        # ---- S1: seq = X @ W_trans + b  (natural layout [1024 tok, 512]) ----
        wtrans_t = []
        for kt in range(KT_H):
            t = T(pp, [128, EMB], SEQ_DT, f"wtrans{kt}")
            dma(t[:], wtrans[kt * 128:(kt + 1) * 128, :])
            wtrans_t.append(t)

        ps_seq = [T(pps, [128, EMB], FP32, "ps") for _ in range(8)]
        for kt in range(KT_H):
            xt_t = T(pst, [128, C], SEQ_DT, "xt_stream", bufs=2)
            dma(xt_t[:], xt[kt * 128:(kt + 1) * 128, :])
            for mt in range(8):
                nc.tensor.matmul(
                    ps_seq[mt][:], xt_t[:, mt * 128:(mt + 1) * 128], wtrans_t[kt][:],
                    start=(kt == 0), stop=(kt == KT_H - 1))
        seq_t = []
        for mt in range(8):
            t = T(pp, [128, EMB], SEQ_DT, f"seq{mt}")
            nc.vector.tensor_add(t[:], ps_seq[mt][:], btrans_bc[:])
            seq_t.append(t)

        # ---- S2: mention embeddings + entity logsumexp nodes ----
        ps_memb = T(pps, [E * M, EMB], FP32, "ps")
        for kt in range(KT_H):
            xg_t = T(pst, [128, E * M], SEQ_DT, "xg_stream", bufs=3)
            dma(xg_t[:], xg[kt * 128:(kt + 1) * 128, :])
            nc.tensor.matmul(ps_memb[:], xg_t[:], wtrans_t[kt][:],
                             start=(kt == 0), stop=(kt == KT_H - 1))
        memb_t = T(pp, [E * M, EMB], FP32, "memb")
        nc.vector.tensor_add(memb_t[:], ps_memb[:], btrans_bc[0:E * M, :])
        nc.vector.tensor_copy(nodes_m[:, 0:EMB], memb_t[:])
        ememb_t = T(pp, [E * M, EMB], FP32, "ememb")
        nc.scalar.activation(ememb_t[:], memb_t[:], ACT.Exp)
        g3_t = T(pp, [E * M, E], FP32, "g3")
        dma(g3_t[:], g3)
        ps_ent = T(pps, [E, EMB], FP32, "ps")
        nc.tensor.matmul(ps_ent[:], g3_t[:], ememb_t[:], start=True, stop=True)
        nc.scalar.activation(nodes_e[:, 0:EMB], ps_ent[:], ACT.Ln)

        # ---- S3: link nodes ----
        # a[s] = mean over (h,i) of the 16x16 link attention block, s=(l,j)
        aT_t, aTb_t, xspan_t, gspan_t = [], [], [], []
        for i, (off, sz) in enumerate(_ts(SPAN_TILES)):
            al = T(pst, [sz, NH * LS], FP32, "attl_stream", bufs=2)
            dma(al[:], attl[off:off + sz, :])
            a = T(pp, [sz, 1], FP32, f"aT{i}")
            nc.vector.tensor_reduce(a[:], al[:], mybir.AxisListType.X,
                                    mybir.AluOpType.add)
            nc.vector.tensor_scalar_mul(a[:], a[:], 1.0 / (NH * LS))
            aT_t.append(a)
            ab = T(pp, [sz, 1], SEQ_DT, f"aTb{i}")
            nc.vector.tensor_copy(ab[:], a[:])
            aTb_t.append(ab)
            gs = T(pp, [sz, L], SEQ_DT, f"gspan{i}")
            dma(gs[:], gspan[off:off + sz, :])
            gspan_t.append(gs)
            xs = T(pp, [sz, H], SEQ_DT, f"xspan{i}")
            dma(xs[:], xspan[off:off + sz, :])
            xspan_t.append(xs)
        # asum[l] = sum_j a_l[j] (for the bias term); uses unscaled-by-X a
        ps_as = T(pps, [L, 1], FP32, "ps")
        for kt in range(4):
            nc.tensor.matmul(ps_as[:], gspan_t[kt][:], aTb_t[kt][:],
                             start=(kt == 0), stop=(kt == 3))
        asum_t = T(pp, [L, 1], FP32, "asum")
        nc.vector.tensor_copy(asum_t[:], ps_as[:])
        # scale xspan rows by a in place, then project through gspan
        for kt in range(4):
            nc.vector.tensor_scalar_mul(xspan_t[kt][:], xspan_t[kt][:],
                                        aT_t[kt][:])
        # linkctxT [768, 30]
        lct_t = []
        for mt in range(KT_H):
            ps = T(pps, [128, L], FP32, "ps")
            for kt in range(4):
                nc.tensor.matmul(ps[:], xspan_t[kt][:, mt * 128:(mt + 1) * 128],
                                 gspan_t[kt][:], start=(kt == 0), stop=(kt == 3))
            t = T(pp, [128, L], SEQ_DT, f"lct{mt}")
            nc.vector.tensor_copy(t[:], ps[:])
            lct_t.append(t)
        bterm_t = T(pp, [L, EMB], FP32, "bterm")
        nc.vector.tensor_scalar_mul(bterm_t[:], btrans_bc[0:L, :], asum_t[:])
        ps_link = T(pps, [L, EMB], FP32, "ps")
        for kt in range(KT_H):
            nc.tensor.matmul(ps_link[:], lct_t[kt][:], wtrans_t[kt][:],
                             start=(kt == 0), stop=(kt == KT_H - 1))
        nc.vector.tensor_add(nodes_l[:, 0:EMB], ps_link[:], bterm_t[:])

        # ---- S4: ea (entity attention) + e_ctx ----
        ps_ea = [T(pps, [E, 512], FP32, "ps") for _ in range(2)]
        n_attm = len(ATTM_TILES)
        for i, (off, sz) in enumerate(_ts(ATTM_TILES)):
            at = T(pst, [sz, C], GRAPH_DT, "attm_stream", bufs=2)
            dma(at[:], attm[off:off + sz, :])
            gt = T(pst, [sz, E], GRAPH_DT, "gmat_stream", bufs=3)
            dma(gt[:], gmat[off:off + sz, :])
            for half in range(2):
                nc.tensor.matmul(ps_ea[half][:], gt[:],
                                 at[:, half * 512:(half + 1) * 512],
                                 start=(i == 0), stop=(i == n_attm - 1))
        ea_t = T(pp, [E, C], FP32, "ea")
        for half in range(2):
            nc.vector.tensor_copy(ea_t[:, half * 512:(half + 1) * 512],
                                  ps_ea[half][:])
        rsum_t = T(pp, [E, 1], FP32, "rsum")
        nc.vector.tensor_reduce(rsum_t[:], ea_t[:], mybir.AxisListType.X,
                                mybir.AluOpType.add)
        nc.vector.tensor_scalar_add(rsum_t[:], rsum_t[:], 1e-5)
        recip_t = T(pp, [E, 1], FP32, "recip")
        nc.vector.reciprocal(recip_t[:], rsum_t[:])
        ean_t = T(pp, [E, C], FP32, "ean")
        nc.vector.tensor_scalar_mul(ean_t[:], ea_t[:], recip_t[:])
        # eaNT via PE transpose, then e_ctx [22, 512]
        ps_ectx = T(pps, [E, EMB], FP32, "ps")
        for kt in range(8):
            pst_ea = T(pps, [128, E], FP32, "ps")
            nc.tensor.transpose(pst_ea[:], ean_t[:, kt * 128:(kt + 1) * 128],
                                ident_t[0:E, 0:E])
            eaT = T(pst, [128, E], SEQ_DT, "eaT_stream", bufs=3)
            nc.vector.tensor_copy(eaT[:], pst_ea[:])
            nc.tensor.matmul(ps_ectx[:], eaT[:], seq_t[kt][:],
                             start=(kt == 0), stop=(kt == 7))
        ectx_t = T(pp, [E, EMB], FP32, "ectx")
        nc.vector.tensor_copy(ectx_t[:], ps_ectx[:])

        def conv_pass(ps_c, in_tiles, w_dram, n_ic_t, n_oc_t, ocs,
                      first, last):
            """One accumulation pass of a 5x5 SAME conv on 22x22."""
            n_acc = 25 * n_ic_t
            a = 0
            for tap in range(25):
                di, dj = divmod(tap, 5)
                for kt in range(n_ic_t):
                    w = T(pst, [128, ocs], CONV_DT, "wconv_stream", bufs=6)
                    dma(w[:], w_dram[(tap * n_ic_t + kt) * 128:
                                     (tap * n_ic_t + kt + 1) * 128, :])
                    rhs = in_tiles[kt][:].rearrange(
                        "p (a b) -> p a b", a=PW, b=PW)[:, di:di + S, dj:dj + S]
                    for mt in range(n_oc_t):
                        nc.tensor.matmul(ps_c[mt][:], w[:, mt * 128:(mt + 1) * 128],
                                         rhs, start=(first and a == 0),
                                         stop=(last and a == n_acc - 1))
                    a += 1

        def conv(in_tiles, w_dram, n_ic_t, n_oc_t, ocs, bias_tiles, out_cb):
            ps_c = [T(pps, [128, SP], FP32, "ps") for _ in range(n_oc_t)]
            conv_pass(ps_c, in_tiles, w_dram, n_ic_t, n_oc_t, ocs, True, True)
            for mt in range(n_oc_t):
                out_cb(mt, ps_c[mt])

        # ectxT tiles [128, 22] x4
        ectxT_t = []
        for mt in range(4):
            ps = T(pps, [128, E], FP32, "ps")
            nc.tensor.transpose(ps[:], ectx_t[:, mt * 128:(mt + 1) * 128],
                                ident_t[0:E, 0:E])
            t = T(pp, [128, E], FP32, f"ectxT{mt}")
            nc.vector.tensor_copy(t[:], ps[:])
            ectxT_t.append(t)

        # ---- S5: RGCN ----
        # nodesT [532, 118] built from the three node-group tiles
        NODE_GROUPS = [(0, E), (E, E * M), (E + E * M, L)]
        node_tiles = [nodes_e, nodes_m, nodes_l]
        # bf16 copies of the node features for the rgcn matmuls
        nodes_bf = []
        for gi, (goff, gsz) in enumerate(NODE_GROUPS):
            t = T(pp, [gsz, NF], GRAPH_DT, f"nodes_bf{gi}")
            nc.vector.tensor_copy(t[:], node_tiles[gi][:])
            nodes_bf.append(t)
        nodesT_t = []
        for i, (off, sz) in enumerate(_ts(NF_TILES)):
            t = T(pp, [sz, NN], GRAPH_DT, f"nodesT{i}")
            for gi, (goff, gsz) in enumerate(NODE_GROUPS):
                ps = T(pps, [sz, 128], FP32, "ps")
                nc.tensor.transpose(ps[0:sz, 0:gsz],
                                    node_tiles[gi][:, off:off + sz],
                                    ident_t[0:gsz, 0:gsz])
                nc.vector.tensor_copy(t[:, goff:goff + gsz], ps[0:sz, 0:gsz])
            nodesT_t.append(t)
        # adjacency normalize; adjn split into the three node row-groups
        adjt_t = T(pp, [NN, 4 * NN], FP32, "adjt")
        dma(adjt_t[:], adjt)
        ps_rs = T(pps, [1, 4 * NN], FP32, "ps")
        nc.tensor.matmul(ps_rs[:], ones_t[0:NN, 0:1], adjt_t[:],
                         start=True, stop=True)
        rs_t = T(pp, [1, 4 * NN], FP32, "rs")
        nc.vector.tensor_scalar_add(rs_t[:], ps_rs[:], 1e-5)
        rcp_t = T(pp, [1, 4 * NN], FP32, "rcp")
        nc.vector.reciprocal(rcp_t[:], rs_t[:])
        rsbc_t = T(pp, [128, 4 * NN], FP32, "rsbc")
        nc.gpsimd.partition_broadcast(rsbc_t[:], rcp_t[:])
        adjn_t = []
        for gi, (goff, gsz) in enumerate(NODE_GROUPS):
            tf = T(pst, [gsz, 4 * NN], FP32, "adjn_f32", bufs=3)
            dma(tf[:], adjt[goff:goff + gsz, :])
            t = T(pp, [gsz, 4 * NN], GRAPH_DT, f"adjn{gi}")
            nc.vector.tensor_mul(t[:], tf[:], rsbc_t[0:gsz, :])
            adjn_t.append(t)

        ps_gcn = [T(pps, [128, NN], FP32, "ps") for _ in range(4)]
        n_terms = 5 * 5  # (4 rel + self) x 5 k-tiles
        term = 0
        for r in range(5):
            # r<4: relation r via msgT; r==4: self term via nodesT
            if r < 4:
                msgT_t = []
                for i, (off, sz) in enumerate(_ts(NF_TILES)):
                    ps = T(pps, [sz, NN], FP32, "ps")
                    for gi, (goff, gsz) in enumerate(NODE_GROUPS):
                        nc.tensor.matmul(ps[:], nodes_bf[gi][:, off:off + sz],
                                         adjn_t[gi][:, r * NN:(r + 1) * NN],
                                         start=(gi == 0), stop=(gi == 2))
                    t = T(pst, [sz, NN], GRAPH_DT, f"msgT_stream{i}", bufs=2)
                    nc.vector.tensor_copy(t[:], ps[:])
                    msgT_t.append(t)
                rhs_t = msgT_t
            else:
                rhs_t = nodesT_t
            for i, (off, sz) in enumerate(_ts(NF_TILES)):
                w = T(pst, [sz, EMB], GRAPH_DT, "wg_stream", bufs=3)
                if r < 4:
                    dma(w[:], wrel[r * NF + off:r * NF + off + sz, :])
                else:
                    dma(w[:], wself[off:off + sz, :])
                for mt in range(4):
                    nc.tensor.matmul(ps_gcn[mt][:], w[:, mt * 128:(mt + 1) * 128],
                                     rhs_t[i][:], start=(term == 0),
                                     stop=(term == n_terms - 1))
                term += 1
        # [512,1] needs 4 partition tiles
        brgcn_tiles = []
        for mt in range(4):
            t = T(pp, [128, 1], FP32, f"brgcn{mt}")
            dma(t[:], brgcn[mt * 128:(mt + 1) * 128, :])
            brgcn_tiles.append(t)
        gcnT_t = []
        for mt in range(4):
            t = T(pp, [128, NN], FP32, f"gcnT{mt}")
            nc.scalar.activation(t[:], ps_gcn[mt][:], ACT.Relu,
                                 bias=brgcn_tiles[mt][:, 0:1])
            gcnT_t.append(t)
        # ent natural [22, 512]; entT view = gcnT[:, 0:22]
        ent_t = T(pp, [E, EMB], FP32, "ent")
        for mt in range(4):
            ps = T(pps, [E, 128], FP32, "ps")
            nc.tensor.transpose(ps[:], gcnT_t[mt][:, 0:E], ident_t[:, :])
            nc.vector.tensor_copy(ent_t[:, mt * 128:(mt + 1) * 128], ps[:])
        # ---- S6: relation map x + conv stack ----
        xpad_t = []
        for mt in range(4):
            xp = T(pp, [128, SPP], CONV_DT, f"xpad{mt}")
            nc.vector.memset(xp[:], 0.0)
            entT_v = gcnT_t[mt][:, 0:E]
            t1 = T(pp, [128, SP], FP32, "xtmp1")
            nc.vector.tensor_mul(
                t1[:].rearrange("p (a b) -> p a b", a=S, b=S),
                entT_v.unsqueeze(2).to_broadcast((128, S, S)),
                entT_v.unsqueeze(1).to_broadcast((128, S, S)))
            t2 = T(pp, [128, SP], FP32, "xtmp2")
            nc.vector.tensor_mul(
                t2[:].rearrange("p (a b) -> p a b", a=S, b=S),
                ectxT_t[mt][:].unsqueeze(2).to_broadcast((128, S, S)),
                ectxT_t[mt][:].unsqueeze(1).to_broadcast((128, S, S)))
            inner = xp[:].rearrange("p (a b) -> p a b", a=PW, b=PW)[:, 2:2 + S, 2:2 + S]
            nc.vector.tensor_add(inner, t1[:], t2[:])
            xpad_t.append(xp)

        # conv1: 512 -> 256, output into padded tiles for conv2
        pad1_t = []
        for mt in range(2):
            t = T(pp, [128, SPP], CONV_DT, f"pad1_{mt}")
            nc.vector.memset(t[:], 0.0)
            pad1_t.append(t)
        b1_tiles = []
        for mt in range(2):
            t = T(pp, [128, 1], FP32, f"b1_{mt}")
            dma(t[:], b1[mt * 128:(mt + 1) * 128, :])
            b1_tiles.append(t)

        def c1_out(mt, ps):
            inner = pad1_t[mt][:].rearrange("p (a b) -> p a b", a=PW, b=PW)[
                :, 2:2 + S, 2:2 + S]
            nc.scalar.activation(inner, ps[:].rearrange("p (a b) -> p a b", a=S, b=S),
                                 ACT.Relu, bias=b1_tiles[mt][:, 0:1])

        conv(xpad_t, w1t, 4, 2, IC, b1_tiles, c1_out)

        pad2_t = []
        for mt in range(2):
            t = T(pp, [128, SPP], CONV_DT, f"pad2_{mt}")
            nc.vector.memset(t[:], 0.0)
            pad2_t.append(t)
        b2_tiles = []
        for mt in range(2):
            t = T(pp, [128, 1], FP32, f"b2_{mt}")
            dma(t[:], b2[mt * 128:(mt + 1) * 128, :])
            b2_tiles.append(t)

        def c2_out(mt, ps):
            inner = pad2_t[mt][:].rearrange("p (a b) -> p a b", a=PW, b=PW)[
                :, 2:2 + S, 2:2 + S]
            nc.scalar.activation(inner, ps[:].rearrange("p (a b) -> p a b", a=S, b=S),
                                 ACT.Relu, bias=b2_tiles[mt][:, 0:1])

        conv(pad1_t, w2t, 2, 2, IC, b2_tiles, c2_out)

        x3_t = []
        b3_tiles = []
        for mt in range(4):
            t = T(pp, [128, 1], FP32, f"b3_{mt}")
            dma(t[:], b3[mt * 128:(mt + 1) * 128, :])
            b3_tiles.append(t)
        for mt in range(4):
            t = T(pp, [128, SP], PAIR_DT, f"x3_{mt}")
            x3_t.append(t)

        def c3_out(mt, ps):
            nc.scalar.activation(x3_t[mt][:], ps[:], ACT.Relu,
                                 bias=b3_tiles[mt][:, 0:1])

        conv(pad2_t, w3t, 2, 4, EMB, b3_tiles, c3_out)

        # ---- S7: pair features + classifier ----
        # x3T [484, 512]
        x3T_t = []
        for i, (off, sz) in enumerate(_ts(SP_TILES)):
            t = T(pp, [sz, EMB], PAIR_DT, f"x3T{i}")
            x3T_t.append(t)
        for i, (off, sz) in enumerate(_ts(SP_TILES)):
            for src in range(4):
                ps = T(pps, [sz, 64], FP32, "ps")
                psb = ps[:].bitcast(PAIR_DT)
                nc.tensor.transpose(psb, x3_t[src][:, off:off + sz],
                                    identp_t[:, :])
                nc.vector.tensor_copy(x3T_t[i][:, src * 128:(src + 1) * 128], psb)

        sh_t = T(pp, [E, PH], FP32, "sh")
        dma(sh_t[:], sh)
        st_t = T(pp, [E, PH], FP32, "st")
        dma(st_t[:], st)
        sm_t = []
        for i, (off, sz) in enumerate(_ts(SP_TILES)):
            t = T(pp, [sz, PH], PAIR_DT, f"sm{i}")
            dma(t[:], sm[off:off + sz, :])
            sm_t.append(t)

        featT = [None] * 16
        for mt in range(4):
            ps = T(pps, [128, PH], FP32, "ps")
            nc.tensor.matmul(ps[:], ent_t[:, mt * 128:(mt + 1) * 128], sh_t[:],
                             start=True, stop=True)
            t = T(pp, [128, PH], PAIR_DT, f"featT{mt}")
            nc.vector.tensor_copy(t[:], ps[:])
            featT[mt] = t
        for mt in range(4):
            ps = T(pps, [128, PH], FP32, "ps")
            nc.tensor.matmul(ps[:], ent_t[:, mt * 128:(mt + 1) * 128], st_t[:],
                             start=True, stop=True)
            t = T(pp, [128, PH], PAIR_DT, f"featT{4 + mt}")
            nc.vector.tensor_copy(t[:], ps[:])
            featT[4 + mt] = t
        for mt in range(4):
            ps = T(pps, [128, PH], FP32, "ps")
            for i, (off, sz) in enumerate(_ts(SP_TILES)):
                nc.tensor.matmul(ps[:], x3T_t[i][:, mt * 128:(mt + 1) * 128],
                                 sm_t[i][:], start=(i == 0), stop=(i == 3))
            t = T(pp, [128, PH], PAIR_DT, f"featT{8 + mt}")
            nc.vector.tensor_copy(t[:], ps[:])
            featT[8 + mt] = t
        for mt in range(4):
            t = T(pp, [128, PH], PAIR_DT, f"featT{12 + mt}")
            nc.vector.tensor_mul(t[:], featT[mt][:], featT[4 + mt][:])
            featT[12 + mt] = t

        bht_tiles = []
        for mt in range(8):
            t = T(pp, [128, 1], FP32, f"bht{mt}")
            dma(t[:], bht[mt * 128:(mt + 1) * 128, :])
            bht_tiles.append(t)
        ps_ht = [T(pps, [128, PH], FP32, "ps") for _ in range(8)]
        for kt in range(16):
            w = T(pst, [128, 2 * EMB], PAIR_DT, "wht_stream", bufs=2)
            dma(w[:], wht[kt * 128:(kt + 1) * 128, :])
            for mt in range(8):
                nc.tensor.matmul(ps_ht[mt][:], w[:, mt * 128:(mt + 1) * 128],
                                 featT[kt][:], start=(kt == 0), stop=(kt == 15))
        htT_t = []
        for mt in range(8):
            t = T(pp, [128, PH], PAIR_DT, f"htT{mt}")
            nc.scalar.activation(t[:], ps_ht[mt][:], ACT.Tanh,
                                 bias=bht_tiles[mt][:, 0:1])
            htT_t.append(t)

        ps_out = T(pps, [97, PH], FP32, "ps")
        for kt in range(8):
            w = T(pst, [128, 97], PAIR_DT, "wbil_stream", bufs=3)
            dma(w[:], wbil[kt * 128:(kt + 1) * 128, :])
            nc.tensor.matmul(ps_out[:], w[:], htT_t[kt][:],
                             start=(kt == 0), stop=(kt == 7))
        bbil_t = T(pp, [97, 1], FP32, "bbil")
        dma(bbil_t[:], bbil)
        out_t = T(pp, [97, PH], FP32, "out")
        nc.vector.tensor_scalar_add(out_t[:], ps_out[:], bbil_t[:, 0:1])
        dma(outt, out_t[:])

    nc.compile()
    return nc


_PROG = None


def _get_prog():
    global _PROG
    if _PROG is None:
        _PROG = build_program()
    return _PROG


def _np(dt):
    return _NPDT[dt]


def _shared_inputs(inputs):
    f32 = np.float32
    sh = {}
    sh["wtrans"] = np.ascontiguousarray(inputs["W_trans"], _np(SEQ_DT))
    sh["btrans"] = np.ascontiguousarray(inputs["b_trans"], f32).reshape(1, EMB)
    sh["gmat"] = np.kron(np.eye(E, dtype=f32),
                         np.ones((M * NH, 1), f32) / (M * NH)).astype(_np(GRAPH_DT))
    sh["g3"] = np.kron(np.eye(E, dtype=f32), np.ones((M, 1), f32))
    sh["gspan"] = np.kron(np.eye(L, dtype=f32), np.ones((LS, 1), f32)).astype(_np(SEQ_DT))
    sh["ones"] = np.ones((128, 1), f32)
    sh["ident"] = np.eye(128, dtype=f32)
    sh["identp"] = np.eye(128, dtype=_np(PAIR_DT))
    sh["wrel"] = np.ascontiguousarray(inputs["W_rel"], f32).reshape(4 * NF, EMB).astype(_np(GRAPH_DT))
    sh["wself"] = np.ascontiguousarray(inputs["W_self"], f32).astype(_np(GRAPH_DT))
    sh["brgcn"] = np.ascontiguousarray(inputs["b_rgcn"], f32).reshape(EMB, 1)
    sh["w1t"] = np.ascontiguousarray(
        np.asarray(inputs["conv1_w"], f32).transpose(2, 3, 1, 0).reshape(25 * EMB, IC),
        _np(CONV_DT))
    sh["b1"] = np.ascontiguousarray(inputs["conv1_b"], f32).reshape(IC, 1)
    sh["w2t"] = np.ascontiguousarray(
        np.asarray(inputs["conv2_w"], f32).transpose(2, 3, 1, 0).reshape(25 * IC, IC),
        _np(CONV_DT))
    sh["b2"] = np.ascontiguousarray(inputs["conv2_b"], f32).reshape(IC, 1)
    sh["w3t"] = np.ascontiguousarray(
        np.asarray(inputs["conv3_w"], f32).transpose(2, 3, 1, 0).reshape(25 * IC, EMB),
        _np(CONV_DT))
    sh["b3"] = np.ascontiguousarray(inputs["conv3_b"], f32).reshape(EMB, 1)
    sh["wht"] = np.ascontiguousarray(inputs["ht_W"], _np(PAIR_DT))
    sh["bht"] = np.ascontiguousarray(inputs["ht_b"], f32).reshape(2 * EMB, 1)
    sh["wbil"] = np.ascontiguousarray(inputs["bil_W"], _np(PAIR_DT))
    sh["bbil"] = np.ascontiguousarray(inputs["bil_b"], f32).reshape(97, 1)
    return sh


def _core_inputs(inputs, shared, b, hh):
    f32 = np.float32
    X = np.asarray(inputs["sequence_output"][b], f32)
    att = np.asarray(inputs["attention"][b], f32)
    adj = np.asarray(inputs["adjacency"][b], f32)
    mf = np.asarray(inputs["mention_idx"][b]).reshape(-1).astype(np.int64)
    ls = np.asarray(inputs["link_start"][b]).reshape(-1).astype(np.int64)
    ntypes = np.asarray(inputs["node_types"][b]).astype(np.int64)
    hts = np.asarray(inputs["hts"][b]).astype(np.int64)

    m = dict(shared)
    m["xt"] = np.ascontiguousarray(X.T, _np(SEQ_DT))
    m["xg"] = np.ascontiguousarray(X[mf].T, _np(SEQ_DT))
    pos = ls[:, None] + np.arange(LS)
    m["xspan"] = np.ascontiguousarray(X[pos.reshape(-1)]).astype(_np(SEQ_DT))
    rows = att[:, mf, :]
    m["attm"] = np.ascontiguousarray(
        rows.transpose(1, 0, 2).reshape(ATTM_ROWS, C)).astype(_np(GRAPH_DT))
    attl = np.empty((SPAN_ROWS, NH * LS), f32)
    for l in range(L):
        blk = att[:, pos[l], :][:, :, pos[l]]           # [12, 16i, 16j]
        attl[l * LS:(l + 1) * LS, :] = blk.transpose(2, 0, 1).reshape(LS, NH * LS)
    m["attl"] = attl
    m["adjt"] = np.ascontiguousarray(
        np.concatenate([adj[r].T for r in range(4)], axis=1), f32)
    m["typ"] = np.ascontiguousarray(
        np.asarray(inputs["type_embed"], f32)[ntypes], f32)
    pr = hts[hh * PH:(hh + 1) * PH]
    shm = np.zeros((E, PH), f32)
    shm[pr[:, 0], np.arange(PH)] = 1.0
    stm = np.zeros((E, PH), f32)
    stm[pr[:, 1], np.arange(PH)] = 1.0
    smm = np.zeros((SP, PH), f32)
    smm[pr[:, 0] * E + pr[:, 1], np.arange(PH)] = 1.0
    m["sh"] = shm
    m["st"] = stm
    m["sm"] = np.ascontiguousarray(smm, _np(PAIR_DT))
    return m


def kernel(**inputs):
    nc = _get_prog()
    shared = _shared_inputs(inputs)
    in_maps = []
    for b in range(B):
        for hh in range(2):
            in_maps.append(_core_inputs(inputs, shared, b, hh))
    res = run_bass_kernel_spmd(nc, in_maps, core_ids=list(range(8)))
    out = np.empty((B, P, 97), np.float32)
    for b in range(B):
        for hh in range(2):
            out[b, hh * PH:(hh + 1) * PH, :] = np.asarray(
                res.results[2 * b + hh]["outt"], np.float32).T
    return out


# revision 14
# speedup vs baseline: 1.2959x; 1.0700x over previous
"""Trainium2 Bass kernel for nn_DocREModel (8-core SPMD).

Sharding: data-parallel over the 4 documents x 2 pair-halves = 8 cores.
Each core runs an identical program; per-core behavior differs only via
its input data (its doc's tensors + its half of the pair one-hots).

All floating-point arithmetic runs on device. Host does only index-driven
data movement: batch slicing, transposes, row gathers at integer indices,
and one-hot/selector matrix construction.
"""

import numpy as np
from contextlib import ExitStack

import concourse.bass as bass
import concourse.bacc as bacc
import concourse.tile as tile
import concourse.mybir as mybir
from concourse.bass_utils import run_bass_kernel_spmd

FP32 = mybir.dt.float32
BF16 = mybir.dt.bfloat16

# compute dtypes per stage
SEQ_DT = BF16    # seq transform matmuls
CONV_DT = BF16   # conv stack
PAIR_DT = BF16   # pair-classification matmuls
GRAPH_DT = BF16  # rgcn / entity-attention matmuls

import ml_dtypes

_NPDT = {FP32: np.float32, BF16: ml_dtypes.bfloat16}

B, C, H, NH = 4, 1024, 768, 12
E, M, L, LS = 22, 3, 30, 16
NN, NF, EMB = 118, 532, 512
P, PH = 462, 231
IC = 256
S = 22            # spatial side of relation map
SP = S * S        # 484
PW = S + 4        # 26 padded side
SPP = PW * PW     # 676
ACT = mybir.ActivationFunctionType
KT_H = H // 128   # 6
ATTM_ROWS = E * M * NH          # 792
ATTM_TILES = [128] * 6 + [24]
SPAN_ROWS = L * LS              # 480
SPAN_TILES = [128, 128, 128, 96]
NF_TILES = [128, 128, 128, 128, 20]   # 532
SP_TILES = [128, 128, 128, 100]       # 484


def _ts(sizes):
    """(offset, size) pairs for a tiling."""
    off = 0
    for sz in sizes:
        yield off, sz
        off += sz


def build_program():
    nc = bacc.Bacc("TRN2", target_bir_lowering=False, debug=False)

    dins = {}

    def din(name, shape, dt=FP32):
        dins[name] = nc.dram_tensor(name, shape, dt, kind="ExternalInput").ap()
        return dins[name]

    xt = din("xt", [H, C], SEQ_DT)            # X.T
    xg = din("xg", [H, E * M], SEQ_DT)        # X rows at mention idx, transposed
    xspan = din("xspan", [SPAN_ROWS, H], SEQ_DT)  # X rows at span positions
    attm = din("attm", [ATTM_ROWS, C], GRAPH_DT)  # attention mention rows, (e,m)*12+h major
    attl = din("attl", [SPAN_ROWS, NH * LS], FP32)  # link blocks, row l*16+j, free (h,i)
    adjt = din("adjt", [NN, 4 * NN], FP32)    # col r*118+i = adjacency[r,i,:]
    typ = din("typ", [NN, 20], FP32)          # type_embed[node_types]
    wtrans = din("wtrans", [H, EMB], SEQ_DT)
    btrans = din("btrans", [1, EMB], FP32)
    gmat = din("gmat", [ATTM_ROWS, E], GRAPH_DT)  # kron(I22, ones(36))/36
    g3 = din("g3", [E * M, E], FP32)          # kron(I22, ones(3))
    gspan = din("gspan", [SPAN_ROWS, L], SEQ_DT)  # kron(I30, ones(16))
    ones = din("ones", [128, 1], FP32)
    ident = din("ident", [128, 128], FP32)
    identp = din("identp", [128, 128], PAIR_DT)
    wrel = din("wrel", [4 * NF, EMB], GRAPH_DT)   # rows r*532+k
    wself = din("wself", [NF, EMB], GRAPH_DT)
    brgcn = din("brgcn", [EMB, 1], FP32)
    w1t = din("w1t", [25 * EMB, IC], CONV_DT)   # rows tap*512+ic
    b1 = din("b1", [IC, 1], FP32)
    w2t = din("w2t", [25 * IC, IC], CONV_DT)
    b2 = din("b2", [IC, 1], FP32)
    w3t = din("w3t", [25 * IC, EMB], CONV_DT)
    b3 = din("b3", [EMB, 1], FP32)
    sh = din("sh", [E, PH], FP32)
    st = din("st", [E, PH], FP32)
    sm = din("sm", [SP, PH], PAIR_DT)
    wht = din("wht", [4 * EMB, 2 * EMB], PAIR_DT)
    bht = din("bht", [2 * EMB, 1], FP32)
    wbil = din("wbil", [2 * EMB, 97], PAIR_DT)
    bbil = din("bbil", [97, 1], FP32)
    outt = nc.dram_tensor("outt", [97, PH], FP32, kind="ExternalOutput").ap()

    with tile.TileContext(nc) as tc, ExitStack() as ctx:
        pp = ctx.enter_context(tc.tile_pool(name="persist", bufs=1))
        pst = ctx.enter_context(tc.tile_pool(name="stream", bufs=1))
        pps = ctx.enter_context(tc.tile_pool(name="psum", bufs=8, space="PSUM"))


        dma = nc.sync.dma_start

        def T(pool, shape, dt, tag, bufs=None):
            return pool.tile(shape, dt, tag=tag, name=tag, bufs=bufs)


        # ---- persistent small tiles ----
        ident_t = T(pp, [128, 128], FP32, "ident")
        dma(ident_t[:], ident)
        identp_t = T(pp, [128, 128], PAIR_DT, "identp")
        dma(identp_t[:], identp)
        ones_t = T(pp, [128, 1], FP32, "ones")
        dma(ones_t[:], ones)
        btrans_t = T(pp, [1, EMB], FP32, "btrans")
        dma(btrans_t[:], btrans)
        btrans_bc = T(pp, [128, EMB], FP32, "btrans_bc")
        nc.gpsimd.partition_broadcast(btrans_bc[:], btrans_t[:])
        nodes_e = T(pp, [E, NF], FP32, "nodes_e")
        nodes_m = T(pp, [E * M, NF], FP32, "nodes_m")
        nodes_l = T(pp, [L, NF], FP32, "nodes_l")
        dma(nodes_e[:, EMB:NF], typ[0:E, :])
        dma(nodes_m[:, EMB:NF], typ[E:E + E * M, :])
        dma(nodes_l[:, EMB:NF], typ[E + E * M:NN, :])

        # ---- S1: seq = X @ W_trans + b  (natural layout [1024 tok, 512]) ----
        wtrans_t = []
        for kt in range(KT_H):
            t = T(pp, [128, EMB], SEQ_DT, f"wtrans{kt}")
            dma(t[:], wtrans[kt * 128:(kt + 1) * 128, :])
            wtrans_t.append(t)

        ps_seq = [T(pps, [128, EMB], FP32, "ps") for _ in range(8)]
        for kt in range(KT_H):
            xt_t = T(pst, [128, C], SEQ_DT, "xt_stream", bufs=2)
            dma(xt_t[:], xt[kt * 128:(kt + 1) * 128, :])
            for mt in range(8):
                nc.tensor.matmul(
                    ps_seq[mt][:], xt_t[:, mt * 128:(mt + 1) * 128], wtrans_t[kt][:],
                    start=(kt == 0), stop=(kt == KT_H - 1))
        seq_t = []
        for mt in range(8):
            t = T(pp, [128, EMB], SEQ_DT, f"seq{mt}")
            nc.vector.tensor_add(t[:], ps_seq[mt][:], btrans_bc[:])
            seq_t.append(t)

        # ---- S2: mention embeddings + entity logsumexp nodes ----
        ps_memb = T(pps, [E * M, EMB], FP32, "ps")
        for kt in range(KT_H):
            xg_t = T(pst, [128, E * M], SEQ_DT, "xg_stream", bufs=3)
            dma(xg_t[:], xg[kt * 128:(kt + 1) * 128, :])
            nc.tensor.matmul(ps_memb[:], xg_t[:], wtrans_t[kt][:],
                             start=(kt == 0), stop=(kt == KT_H - 1))
        memb_t = T(pp, [E * M, EMB], FP32, "memb")
        nc.vector.tensor_add(memb_t[:], ps_memb[:], btrans_bc[0:E * M, :])
        nc.vector.tensor_copy(nodes_m[:, 0:EMB], memb_t[:])
        ememb_t = T(pp, [E * M, EMB], FP32, "ememb")
        nc.scalar.activation(ememb_t[:], memb_t[:], ACT.Exp)
        g3_t = T(pp, [E * M, E], FP32, "g3")
        dma(g3_t[:], g3)
        ps_ent = T(pps, [E, EMB], FP32, "ps")
        nc.tensor.matmul(ps_ent[:], g3_t[:], ememb_t[:], start=True, stop=True)
        nc.scalar.activation(nodes_e[:, 0:EMB], ps_ent[:], ACT.Ln)

        # ---- S3: link nodes ----
        # a[s] = mean over (h,i) of the 16x16 link attention block, s=(l,j)
        aT_t, aTb_t, xspan_t, gspan_t = [], [], [], []
        for i, (off, sz) in enumerate(_ts(SPAN_TILES)):
            al = T(pst, [sz, NH * LS], FP32, "attl_stream", bufs=2)
            dma(al[:], attl[off:off + sz, :])
            a = T(pp, [sz, 1], FP32, f"aT{i}")
            nc.vector.tensor_reduce(a[:], al[:], mybir.AxisListType.X,
                                    mybir.AluOpType.add)
            nc.vector.tensor_scalar_mul(a[:], a[:], 1.0 / (NH * LS))
            aT_t.append(a)
            ab = T(pp, [sz, 1], SEQ_DT, f"aTb{i}")
            nc.vector.tensor_copy(ab[:], a[:])
            aTb_t.append(ab)
            gs = T(pp, [sz, L], SEQ_DT, f"gspan{i}")
            dma(gs[:], gspan[off:off + sz, :])
            gspan_t.append(gs)
            xs = T(pp, [sz, H], SEQ_DT, f"xspan{i}")
            dma(xs[:], xspan[off:off + sz, :])
            xspan_t.append(xs)
        # asum[l] = sum_j a_l[j] (for the bias term); uses unscaled-by-X a
        ps_as = T(pps, [L, 1], FP32, "ps")
        for kt in range(4):
            nc.tensor.matmul(ps_as[:], gspan_t[kt][:], aTb_t[kt][:],
                             start=(kt == 0), stop=(kt == 3))
        asum_t = T(pp, [L, 1], FP32, "asum")
        nc.vector.tensor_copy(asum_t[:], ps_as[:])
        # scale xspan rows by a in place, then project through gspan
        for kt in range(4):
            nc.vector.tensor_scalar_mul(xspan_t[kt][:], xspan_t[kt][:],
                                        aT_t[kt][:])
        # linkctxT [768, 30]
        lct_t = []
        for mt in range(KT_H):
            ps = T(pps, [128, L], FP32, "ps")
            for kt in range(4):
                nc.tensor.matmul(ps[:], xspan_t[kt][:, mt * 128:(mt + 1) * 128],
                                 gspan_t[kt][:], start=(kt == 0), stop=(kt == 3))
            t = T(pp, [128, L], SEQ_DT, f"lct{mt}")
            nc.vector.tensor_copy(t[:], ps[:])
            lct_t.append(t)
        bterm_t = T(pp, [L, EMB], FP32, "bterm")
        nc.vector.tensor_scalar_mul(bterm_t[:], btrans_bc[0:L, :], asum_t[:])
        ps_link = T(pps, [L, EMB], FP32, "ps")
        for kt in range(KT_H):
            nc.tensor.matmul(ps_link[:], lct_t[kt][:], wtrans_t[kt][:],
                             start=(kt == 0), stop=(kt == KT_H - 1))
        nc.vector.tensor_add(nodes_l[:, 0:EMB], ps_link[:], bterm_t[:])

        # ---- S4: ea (entity attention) + e_ctx ----
        ps_ea = [T(pps, [E, 512], FP32, "ps") for _ in range(2)]
        n_attm = len(ATTM_TILES)
        for i, (off, sz) in enumerate(_ts(ATTM_TILES)):
            at = T(pst, [sz, C], GRAPH_DT, "attm_stream", bufs=2)
            dma(at[:], attm[off:off + sz, :])
            gt = T(pst, [sz, E], GRAPH_DT, "gmat_stream", bufs=3)
            dma(gt[:], gmat[off:off + sz, :])
            for half in range(2):
                nc.tensor.matmul(ps_ea[half][:], gt[:],
                                 at[:, half * 512:(half + 1) * 512],
                                 start=(i == 0), stop=(i == n_attm - 1))
        ea_t = T(pp, [E, C], FP32, "ea")
        for half in range(2):
            nc.vector.tensor_copy(ea_t[:, half * 512:(half + 1) * 512],
                                  ps_ea[half][:])
        rsum_t = T(pp, [E, 1], FP32, "rsum")
        nc.vector.tensor_reduce(rsum_t[:], ea_t[:], mybir.AxisListType.X,
                                mybir.AluOpType.add)
        nc.vector.tensor_scalar_add(rsum_t[:], rsum_t[:], 1e-5)
        recip_t = T(pp, [E, 1], FP32, "recip")
        nc.vector.reciprocal(recip_t[:], rsum_t[:])
        ean_t = T(pp, [E, C], FP32, "ean")
        nc.vector.tensor_scalar_mul(ean_t[:], ea_t[:], recip_t[:])
        # eaNT via PE transpose, then e_ctx [22, 512]
        ps_ectx = T(pps, [E, EMB], FP32, "ps")
        for kt in range(8):
            pst_ea = T(pps, [128, E], FP32, "ps")
            nc.tensor.transpose(pst_ea[:], ean_t[:, kt * 128:(kt + 1) * 128],
                                ident_t[0:E, 0:E])
            eaT = T(pst, [128, E], SEQ_DT, "eaT_stream", bufs=3)
            nc.vector.tensor_copy(eaT[:], pst_ea[:])
            nc.tensor.matmul(ps_ectx[:], eaT[:], seq_t[kt][:],
                             start=(kt == 0), stop=(kt == 7))
        ectx_t = T(pp, [E, EMB], FP32, "ectx")
        nc.vector.tensor_copy(ectx_t[:], ps_ectx[:])

        def conv_pass(ps_c, in_tiles, w_dram, n_ic_t, n_oc_t, ocs,
                      first, last):
            """One accumulation pass of a 5x5 SAME conv on 22x22."""
            n_acc = 25 * n_ic_t
            a = 0
            for tap in range(25):
                di, dj = divmod(tap, 5)
                for kt in range(n_ic_t):
                    w = T(pst, [128, ocs], CONV_DT, "wconv_stream", bufs=6)
                    dma(w[:], w_dram[(tap * n_ic_t + kt) * 128:
                                     (tap * n_ic_t + kt + 1) * 128, :])
                    rhs = in_tiles[kt][:].rearrange(
                        "p (a b) -> p a b", a=PW, b=PW)[:, di:di + S, dj:dj + S]
                    for mt in range(n_oc_t):
                        nc.tensor.matmul(ps_c[mt][:], w[:, mt * 128:(mt + 1) * 128],
                                         rhs, start=(first and a == 0),
                                         stop=(last and a == n_acc - 1))
                    a += 1

        def conv(in_tiles, w_dram, n_ic_t, n_oc_t, ocs, bias_tiles, out_cb):
            ps_c = [T(pps, [128, SP], FP32, "ps") for _ in range(n_oc_t)]
            conv_pass(ps_c, in_tiles, w_dram, n_ic_t, n_oc_t, ocs, True, True)
            for mt in range(n_oc_t):
                out_cb(mt, ps_c[mt])

        # ---- S5: RGCN ----
        # nodesT [532, 118] built from the three node-group tiles
        NODE_GROUPS = [(0, E), (E, E * M), (E + E * M, L)]
        node_tiles = [nodes_e, nodes_m, nodes_l]
        # bf16 copies of the node features for the rgcn matmuls
        nodes_bf = []
        for gi, (goff, gsz) in enumerate(NODE_GROUPS):
            t = T(pp, [gsz, NF], GRAPH_DT, f"nodes_bf{gi}")
            nc.vector.tensor_copy(t[:], node_tiles[gi][:])
            nodes_bf.append(t)
        nodesT_t = []
        for i, (off, sz) in enumerate(_ts(NF_TILES)):
            t = T(pp, [sz, NN], GRAPH_DT, f"nodesT{i}")
            for gi, (goff, gsz) in enumerate(NODE_GROUPS):
                ps = T(pps, [sz, 128], FP32, "ps")
                nc.tensor.transpose(ps[0:sz, 0:gsz],
                                    node_tiles[gi][:, off:off + sz],
                                    ident_t[0:gsz, 0:gsz])
                nc.vector.tensor_copy(t[:, goff:goff + gsz], ps[0:sz, 0:gsz])
            nodesT_t.append(t)
        # adjacency normalize; adjn split into the three node row-groups
        adjt_t = T(pp, [NN, 4 * NN], FP32, "adjt")
        dma(adjt_t[:], adjt)
        ps_rs = T(pps, [1, 4 * NN], FP32, "ps")
        nc.tensor.matmul(ps_rs[:], ones_t[0:NN, 0:1], adjt_t[:],
                         start=True, stop=True)
        rs_t = T(pp, [1, 4 * NN], FP32, "rs")
        nc.vector.tensor_scalar_add(rs_t[:], ps_rs[:], 1e-5)
        rcp_t = T(pp, [1, 4 * NN], FP32, "rcp")
        nc.vector.reciprocal(rcp_t[:], rs_t[:])
        rsbc_t = T(pp, [128, 4 * NN], FP32, "rsbc")
        nc.gpsimd.partition_broadcast(rsbc_t[:], rcp_t[:])
        adjn_t = []
        for gi, (goff, gsz) in enumerate(NODE_GROUPS):
            tf = T(pst, [gsz, 4 * NN], FP32, "adjn_f32", bufs=3)
            dma(tf[:], adjt[goff:goff + gsz, :])
            t = T(pp, [gsz, 4 * NN], GRAPH_DT, f"adjn{gi}")
            nc.vector.tensor_mul(t[:], tf[:], rsbc_t[0:gsz, :])
            adjn_t.append(t)

        ps_gcn = [T(pps, [128, NN], FP32, "ps") for _ in range(4)]
        n_terms = 5 * 5  # (4 rel + self) x 5 k-tiles
        term = 0
        for r in range(5):
            # r<4: relation r via msgT; r==4: self term via nodesT
            if r < 4:
                msgT_t = []
                for i, (off, sz) in enumerate(_ts(NF_TILES)):
                    ps = T(pps, [sz, NN], FP32, "ps")
                    for gi, (goff, gsz) in enumerate(NODE_GROUPS):
                        nc.tensor.matmul(ps[:], nodes_bf[gi][:, off:off + sz],
                                         adjn_t[gi][:, r * NN:(r + 1) * NN],
                                         start=(gi == 0), stop=(gi == 2))
                    t = T(pst, [sz, NN], GRAPH_DT, f"msgT_stream{i}", bufs=2)
                    nc.vector.tensor_copy(t[:], ps[:])
                    msgT_t.append(t)
                rhs_t = msgT_t
            else:
                rhs_t = nodesT_t
            for i, (off, sz) in enumerate(_ts(NF_TILES)):
                w = T(pst, [sz, EMB], GRAPH_DT, "wg_stream", bufs=3)
                if r < 4:
                    dma(w[:], wrel[r * NF + off:r * NF + off + sz, :])
                else:
                    dma(w[:], wself[off:off + sz, :])
                for mt in range(4):
                    nc.tensor.matmul(ps_gcn[mt][:], w[:, mt * 128:(mt + 1) * 128],
                                     rhs_t[i][:], start=(term == 0),
                                     stop=(term == n_terms - 1))
                term += 1
        # [512,1] needs 4 partition tiles
        brgcn_tiles = []
        for mt in range(4):
            t = T(pp, [128, 1], FP32, f"brgcn{mt}")
            dma(t[:], brgcn[mt * 128:(mt + 1) * 128, :])
            brgcn_tiles.append(t)
        gcnT_t = []
        for mt in range(4):
            t = T(pp, [128, NN], FP32, f"gcnT{mt}")
            nc.scalar.activation(t[:], ps_gcn[mt][:], ACT.Relu,
                                 bias=brgcn_tiles[mt][:, 0:1])
            gcnT_t.append(t)
        # ent natural [22, 512]; entT view = gcnT[:, 0:22]
        ent_t = T(pp, [E, EMB], FP32, "ent")
        for mt in range(4):
            ps = T(pps, [E, 128], FP32, "ps")
            nc.tensor.transpose(ps[:], gcnT_t[mt][:, 0:E], ident_t[:, :])
            nc.vector.tensor_copy(ent_t[:, mt * 128:(mt + 1) * 128], ps[:])
        # ectxT tiles [128, 22] x4
        ectxT_t = []
        for mt in range(4):
            ps = T(pps, [128, E], FP32, "ps")
            nc.tensor.transpose(ps[:], ectx_t[:, mt * 128:(mt + 1) * 128],
                                ident_t[0:E, 0:E])
            t = T(pp, [128, E], FP32, f"ectxT{mt}")
            nc.vector.tensor_copy(t[:], ps[:])
            ectxT_t.append(t)

        # ---- S6: relation map x + conv stack ----
        xpad_t = []
        for mt in range(4):
            xp = T(pp, [128, SPP], CONV_DT, f"xpad{mt}")
            nc.vector.memset(xp[:], 0.0)
            entT_v = gcnT_t[mt][:, 0:E]
            t1 = T(pp, [128, SP], FP32, "xtmp1")
            nc.vector.tensor_mul(
                t1[:].rearrange("p (a b) -> p a b", a=S, b=S),
                entT_v.unsqueeze(2).to_broadcast((128, S, S)),
                entT_v.unsqueeze(1).to_broadcast((128, S, S)))
            t2 = T(pp, [128, SP], FP32, "xtmp2")
            nc.vector.tensor_mul(
                t2[:].rearrange("p (a b) -> p a b", a=S, b=S),
                ectxT_t[mt][:].unsqueeze(2).to_broadcast((128, S, S)),
                ectxT_t[mt][:].unsqueeze(1).to_broadcast((128, S, S)))
            inner = xp[:].rearrange("p (a b) -> p a b", a=PW, b=PW)[:, 2:2 + S, 2:2 + S]
            nc.vector.tensor_add(inner, t1[:], t2[:])
            xpad_t.append(xp)

        # conv1: 512 -> 256, output into padded tiles for conv2
        pad1_t = []
        for mt in range(2):
            t = T(pp, [128, SPP], CONV_DT, f"pad1_{mt}")
            nc.vector.memset(t[:], 0.0)
            pad1_t.append(t)
        b1_tiles = []
        for mt in range(2):
            t = T(pp, [128, 1], FP32, f"b1_{mt}")
            dma(t[:], b1[mt * 128:(mt + 1) * 128, :])
            b1_tiles.append(t)

        def c1_out(mt, ps):
            inner = pad1_t[mt][:].rearrange("p (a b) -> p a b", a=PW, b=PW)[
                :, 2:2 + S, 2:2 + S]
            nc.scalar.activation(inner, ps[:].rearrange("p (a b) -> p a b", a=S, b=S),
                                 ACT.Relu, bias=b1_tiles[mt][:, 0:1])

        conv(xpad_t, w1t, 4, 2, IC, b1_tiles, c1_out)

        pad2_t = []
        for mt in range(2):
            t = T(pp, [128, SPP], CONV_DT, f"pad2_{mt}")
            nc.vector.memset(t[:], 0.0)
            pad2_t.append(t)
        b2_tiles = []
        for mt in range(2):
            t = T(pp, [128, 1], FP32, f"b2_{mt}")
            dma(t[:], b2[mt * 128:(mt + 1) * 128, :])
            b2_tiles.append(t)

        def c2_out(mt, ps):
            inner = pad2_t[mt][:].rearrange("p (a b) -> p a b", a=PW, b=PW)[
                :, 2:2 + S, 2:2 + S]
            nc.scalar.activation(inner, ps[:].rearrange("p (a b) -> p a b", a=S, b=S),
                                 ACT.Relu, bias=b2_tiles[mt][:, 0:1])

        conv(pad1_t, w2t, 2, 2, IC, b2_tiles, c2_out)

        x3_t = []
        b3_tiles = []
        for mt in range(4):
            t = T(pp, [128, 1], FP32, f"b3_{mt}")
            dma(t[:], b3[mt * 128:(mt + 1) * 128, :])
            b3_tiles.append(t)
        for mt in range(4):
            t = T(pp, [128, SP], PAIR_DT, f"x3_{mt}")
            x3_t.append(t)

        def c3_out(mt, ps):
            nc.scalar.activation(x3_t[mt][:], ps[:], ACT.Relu,
                                 bias=b3_tiles[mt][:, 0:1])

        conv(pad2_t, w3t, 2, 4, EMB, b3_tiles, c3_out)

        # ---- S7: pair features + classifier ----
        # x3T [484, 512]
        x3T_t = []
        for i, (off, sz) in enumerate(_ts(SP_TILES)):
            t = T(pp, [sz, EMB], PAIR_DT, f"x3T{i}")
            x3T_t.append(t)
        for i, (off, sz) in enumerate(_ts(SP_TILES)):
            for src in range(4):
                ps = T(pps, [sz, 64], FP32, "ps")
                psb = ps[:].bitcast(PAIR_DT)
                nc.tensor.transpose(psb, x3_t[src][:, off:off + sz],
                                    identp_t[:, :])
                nc.vector.tensor_copy(x3T_t[i][:, src * 128:(src + 1) * 128], psb)

        sh_t = T(pp, [E, PH], FP32, "sh")
        dma(sh_t[:], sh)
        st_t = T(pp, [E, PH], FP32, "st")
        dma(st_t[:], st)
        sm_t = []
        for i, (off, sz) in enumerate(_ts(SP_TILES)):
            t = T(pp, [sz, PH], PAIR_DT, f"sm{i}")
            dma(t[:], sm[off:off + sz, :])
            sm_t.append(t)

        featT = [None] * 16
        for mt in range(4):
            ps = T(pps, [128, PH], FP32, "ps")
            nc.tensor.matmul(ps[:], ent_t[:, mt * 128:(mt + 1) * 128], sh_t[:],
                             start=True, stop=True)
            t = T(pp, [128, PH], PAIR_DT, f"featT{mt}")
            nc.vector.tensor_copy(t[:], ps[:])
            featT[mt] = t
        for mt in range(4):
            ps = T(pps, [128, PH], FP32, "ps")
            nc.tensor.matmul(ps[:], ent_t[:, mt * 128:(mt + 1) * 128], st_t[:],
                             start=True, stop=True)
            t = T(pp, [128, PH], PAIR_DT, f"featT{4 + mt}")
            nc.vector.tensor_copy(t[:], ps[:])
            featT[4 + mt] = t
        for mt in range(4):
            ps = T(pps, [128, PH], FP32, "ps")
            for i, (off, sz) in enumerate(_ts(SP_TILES)):
                nc.tensor.matmul(ps[:], x3T_t[i][:, mt * 128:(mt + 1) * 128],
                                 sm_t[i][:], start=(i == 0), stop=(i == 3))
            t = T(pp, [128, PH], PAIR_DT, f"featT{8 + mt}")
            nc.vector.tensor_copy(t[:], ps[:])
            featT[8 + mt] = t
        for mt in range(4):
            t = T(pp, [128, PH], PAIR_DT, f"featT{12 + mt}")
            nc.vector.tensor_mul(t[:], featT[mt][:], featT[4 + mt][:])
            featT[12 + mt] = t

        bht_tiles = []
        for mt in range(8):
            t = T(pp, [128, 1], FP32, f"bht{mt}")
            dma(t[:], bht[mt * 128:(mt + 1) * 128, :])
            bht_tiles.append(t)
        ps_ht = [T(pps, [128, PH], FP32, "ps") for _ in range(8)]
        for kt in range(16):
            w = T(pst, [128, 2 * EMB], PAIR_DT, "wht_stream", bufs=2)
            dma(w[:], wht[kt * 128:(kt + 1) * 128, :])
            for mt in range(8):
                nc.tensor.matmul(ps_ht[mt][:], w[:, mt * 128:(mt + 1) * 128],
                                 featT[kt][:], start=(kt == 0), stop=(kt == 15))
        htT_t = []
        for mt in range(8):
            t = T(pp, [128, PH], PAIR_DT, f"htT{mt}")
            nc.scalar.activation(t[:], ps_ht[mt][:], ACT.Tanh,
                                 bias=bht_tiles[mt][:, 0:1])
            htT_t.append(t)

        ps_out = T(pps, [97, PH], FP32, "ps")
        for kt in range(8):
            w = T(pst, [128, 97], PAIR_DT, "wbil_stream", bufs=3)
            dma(w[:], wbil[kt * 128:(kt + 1) * 128, :])
            nc.tensor.matmul(ps_out[:], w[:], htT_t[kt][:],
                             start=(kt == 0), stop=(kt == 7))
        bbil_t = T(pp, [97, 1], FP32, "bbil")
        dma(bbil_t[:], bbil)
        out_t = T(pp, [97, PH], FP32, "out")
        nc.vector.tensor_scalar_add(out_t[:], ps_out[:], bbil_t[:, 0:1])
        dma(outt, out_t[:])

    nc.compile()
    return nc


_PROG = None


def _get_prog():
    global _PROG
    if _PROG is None:
        _PROG = build_program()
    return _PROG


def _np(dt):
    return _NPDT[dt]


def _shared_inputs(inputs):
    f32 = np.float32
    sh = {}
    sh["wtrans"] = np.ascontiguousarray(inputs["W_trans"], _np(SEQ_DT))
    sh["btrans"] = np.ascontiguousarray(inputs["b_trans"], f32).reshape(1, EMB)
    sh["gmat"] = np.kron(np.eye(E, dtype=f32),
                         np.ones((M * NH, 1), f32) / (M * NH)).astype(_np(GRAPH_DT))
    sh["g3"] = np.kron(np.eye(E, dtype=f32), np.ones((M, 1), f32))
    sh["gspan"] = np.kron(np.eye(L, dtype=f32), np.ones((LS, 1), f32)).astype(_np(SEQ_DT))
    sh["ones"] = np.ones((128, 1), f32)
    sh["ident"] = np.eye(128, dtype=f32)
    sh["identp"] = np.eye(128, dtype=_np(PAIR_DT))
    sh["wrel"] = np.ascontiguousarray(inputs["W_rel"], f32).reshape(4 * NF, EMB).astype(_np(GRAPH_DT))
    sh["wself"] = np.ascontiguousarray(inputs["W_self"], f32).astype(_np(GRAPH_DT))
    sh["brgcn"] = np.ascontiguousarray(inputs["b_rgcn"], f32).reshape(EMB, 1)
    sh["w1t"] = np.ascontiguousarray(
        np.asarray(inputs["conv1_w"], f32).transpose(2, 3, 1, 0).reshape(25 * EMB, IC),
        _np(CONV_DT))
    sh["b1"] = np.ascontiguousarray(inputs["conv1_b"], f32).reshape(IC, 1)
    sh["w2t"] = np.ascontiguousarray(
        np.asarray(inputs["conv2_w"], f32).transpose(2, 3, 1, 0).reshape(25 * IC, IC),
        _np(CONV_DT))
    sh["b2"] = np.ascontiguousarray(inputs["conv2_b"], f32).reshape(IC, 1)
    sh["w3t"] = np.ascontiguousarray(
        np.asarray(inputs["conv3_w"], f32).transpose(2, 3, 1, 0).reshape(25 * IC, EMB),
        _np(CONV_DT))
    sh["b3"] = np.ascontiguousarray(inputs["conv3_b"], f32).reshape(EMB, 1)
    sh["wht"] = np.ascontiguousarray(inputs["ht_W"], _np(PAIR_DT))
    sh["bht"] = np.ascontiguousarray(inputs["ht_b"], f32).reshape(2 * EMB, 1)
    sh["wbil"] = np.ascontiguousarray(inputs["bil_W"], _np(PAIR_DT))
    sh["bbil"] = np.ascontiguousarray(inputs["bil_b"], f32).reshape(97, 1)
    return sh


def _core_inputs(inputs, shared, b, hh):
    f32 = np.float32
    X = np.asarray(inputs["sequence_output"][b], f32)
    att = np.asarray(inputs["attention"][b], f32)
    adj = np.asarray(inputs["adjacency"][b], f32)
    mf = np.asarray(inputs["mention_idx"][b]).reshape(-1).astype(np.int64)
    ls = np.asarray(inputs["link_start"][b]).reshape(-1).astype(np.int64)
    ntypes = np.asarray(inputs["node_types"][b]).astype(np.int64)
    hts = np.asarray(inputs["hts"][b]).astype(np.int64)

    m = dict(shared)
    m["xt"] = np.ascontiguousarray(X.T, _np(SEQ_DT))
    m["xg"] = np.ascontiguousarray(X[mf].T, _np(SEQ_DT))
    pos = ls[:, None] + np.arange(LS)
    m["xspan"] = np.ascontiguousarray(X[pos.reshape(-1)]).astype(_np(SEQ_DT))
    rows = att[:, mf, :]
    m["attm"] = np.ascontiguousarray(
        rows.transpose(1, 0, 2).reshape(ATTM_ROWS, C)).astype(_np(GRAPH_DT))
    attl = np.empty((SPAN_ROWS, NH * LS), f32)
    for l in range(L):
        blk = att[:, pos[l], :][:, :, pos[l]]           # [12, 16i, 16j]
        attl[l * LS:(l + 1) * LS, :] = blk.transpose(2, 0, 1).reshape(LS, NH * LS)
    m["attl"] = attl
    m["adjt"] = np.ascontiguousarray(
        np.concatenate([adj[r].T for r in range(4)], axis=1), f32)
    m["typ"] = np.ascontiguousarray(
        np.asarray(inputs["type_embed"], f32)[ntypes], f32)
    pr = hts[hh * PH:(hh + 1) * PH]
    shm = np.zeros((E, PH), f32)
    shm[pr[:, 0], np.arange(PH)] = 1.0
    stm = np.zeros((E, PH), f32)
    stm[pr[:, 1], np.arange(PH)] = 1.0
    smm = np.zeros((SP, PH), f32)
    smm[pr[:, 0] * E + pr[:, 1], np.arange(PH)] = 1.0
    m["sh"] = shm
    m["st"] = stm
    m["sm"] = np.ascontiguousarray(smm, _np(PAIR_DT))
    return m


def kernel(**inputs):
    nc = _get_prog()
    shared = _shared_inputs(inputs)
    in_maps = []
    for b in range(B):
        for hh in range(2):
            in_maps.append(_core_inputs(inputs, shared, b, hh))
    res = run_bass_kernel_spmd(nc, in_maps, core_ids=list(range(8)))
    out = np.empty((B, P, 97), np.float32)
    for b in range(B):
        for hh in range(2):
            out[b, hh * PH:(hh + 1) * PH, :] = np.asarray(
                res.results[2 * b + hh]["outt"], np.float32).T
    return out


# revision 15
# speedup vs baseline: 1.3884x; 1.0714x over previous
"""Trainium2 Bass kernel for nn_DocREModel (8-core SPMD).

Sharding: data-parallel over the 4 documents x 2 pair-halves = 8 cores.
Each core runs an identical program; per-core behavior differs only via
its input data (its doc's tensors + its half of the pair one-hots).

All floating-point arithmetic runs on device. Host does only index-driven
data movement: batch slicing, transposes, row gathers at integer indices,
and one-hot/selector matrix construction.
"""

import numpy as np
from contextlib import ExitStack

import concourse.bass as bass
import concourse.bacc as bacc
import concourse.tile as tile
import concourse.mybir as mybir
from concourse.bass_utils import run_bass_kernel_spmd

FP32 = mybir.dt.float32
BF16 = mybir.dt.bfloat16

# compute dtypes per stage
SEQ_DT = BF16    # seq transform matmuls
CONV_DT = BF16   # conv stack
PAIR_DT = BF16   # pair-classification matmuls
GRAPH_DT = BF16  # rgcn / entity-attention matmuls

import ml_dtypes

_NPDT = {FP32: np.float32, BF16: ml_dtypes.bfloat16}

B, C, H, NH = 4, 1024, 768, 12
E, M, L, LS = 22, 3, 30, 16
NN, NF, EMB = 118, 532, 512
P, PH = 462, 231
IC = 256
S = 22            # spatial side of relation map
SP = S * S        # 484
PW = 32           # padded side (32 for aligned rows)
SPP = PW * PW     # 676
ACT = mybir.ActivationFunctionType
KT_H = H // 128   # 6
ATTM_ROWS = E * M * NH          # 792
ATTM_TILES = [128] * 6 + [24]
SPAN_ROWS = L * LS              # 480
SPAN_TILES = [128, 128, 128, 96]
NF_TILES = [128, 128, 128, 128, 20]   # 532
SP_TILES = [128, 128, 128, 100]       # 484


def _ts(sizes):
    """(offset, size) pairs for a tiling."""
    off = 0
    for sz in sizes:
        yield off, sz
        off += sz


def build_program():
    nc = bacc.Bacc("TRN2", target_bir_lowering=False, debug=False)

    dins = {}

    def din(name, shape, dt=FP32):
        dins[name] = nc.dram_tensor(name, shape, dt, kind="ExternalInput").ap()
        return dins[name]

    xt = din("xt", [H, C], SEQ_DT)            # X.T
    xg = din("xg", [H, E * M], SEQ_DT)        # X rows at mention idx, transposed
    xspan = din("xspan", [SPAN_ROWS, H], SEQ_DT)  # X rows at span positions
    attm = din("attm", [ATTM_ROWS, C], GRAPH_DT)  # attention mention rows, (e,m)*12+h major
    attl = din("attl", [SPAN_ROWS, NH * LS], FP32)  # link blocks, row l*16+j, free (h,i)
    adjt = din("adjt", [NN, 4 * NN], FP32)    # col r*118+i = adjacency[r,i,:]
    typ = din("typ", [NN, 20], FP32)          # type_embed[node_types]
    wtrans = din("wtrans", [H, EMB], SEQ_DT)
    btrans = din("btrans", [1, EMB], FP32)
    gmat = din("gmat", [ATTM_ROWS, E], GRAPH_DT)  # kron(I22, ones(36))/36
    g3 = din("g3", [E * M, E], FP32)          # kron(I22, ones(3))
    gspan = din("gspan", [SPAN_ROWS, L], SEQ_DT)  # kron(I30, ones(16))
    ones = din("ones", [128, 1], FP32)
    ident = din("ident", [128, 128], FP32)
    identp = din("identp", [128, 128], PAIR_DT)
    wrel = din("wrel", [4 * NF, EMB], GRAPH_DT)   # rows r*532+k
    wself = din("wself", [NF, EMB], GRAPH_DT)
    brgcn = din("brgcn", [EMB, 1], FP32)
    w1t = din("w1t", [25 * EMB, IC], CONV_DT)   # rows tap*512+ic
    b1 = din("b1", [IC, 1], FP32)
    w2t = din("w2t", [25 * IC, IC], CONV_DT)
    b2 = din("b2", [IC, 1], FP32)
    w3t = din("w3t", [25 * IC, EMB], CONV_DT)
    b3 = din("b3", [EMB, 1], FP32)
    sh = din("sh", [E, PH], FP32)
    st = din("st", [E, PH], FP32)
    sm = din("sm", [SP, PH], PAIR_DT)
    wht = din("wht", [4 * EMB, 2 * EMB], PAIR_DT)
    bht = din("bht", [2 * EMB, 1], FP32)
    wbil = din("wbil", [2 * EMB, 97], PAIR_DT)
    bbil = din("bbil", [97, 1], FP32)
    outt = nc.dram_tensor("outt", [97, PH], FP32, kind="ExternalOutput").ap()

    with tile.TileContext(nc) as tc, ExitStack() as ctx:
        pp = ctx.enter_context(tc.tile_pool(name="persist", bufs=1))
        pst = ctx.enter_context(tc.tile_pool(name="stream", bufs=1))
        pps = ctx.enter_context(tc.tile_pool(name="psum", bufs=8, space="PSUM"))


        dma = nc.sync.dma_start

        def T(pool, shape, dt, tag, bufs=None):
            return pool.tile(shape, dt, tag=tag, name=tag, bufs=bufs)


        # ---- persistent small tiles ----
        ident_t = T(pp, [128, 128], FP32, "ident")
        dma(ident_t[:], ident)
        identp_t = T(pp, [128, 128], PAIR_DT, "identp")
        dma(identp_t[:], identp)
        ones_t = T(pp, [128, 1], FP32, "ones")
        dma(ones_t[:], ones)
        btrans_t = T(pp, [1, EMB], FP32, "btrans")
        dma(btrans_t[:], btrans)
        btrans_bc = T(pp, [128, EMB], FP32, "btrans_bc")
        nc.gpsimd.partition_broadcast(btrans_bc[:], btrans_t[:])
        nodes_e = T(pp, [E, NF], FP32, "nodes_e")
        nodes_m = T(pp, [E * M, NF], FP32, "nodes_m")
        nodes_l = T(pp, [L, NF], FP32, "nodes_l")
        dma(nodes_e[:, EMB:NF], typ[0:E, :])
        dma(nodes_m[:, EMB:NF], typ[E:E + E * M, :])
        dma(nodes_l[:, EMB:NF], typ[E + E * M:NN, :])

        # ---- S1: seq = X @ W_trans + b  (natural layout [1024 tok, 512]) ----
        wtrans_t = []
        for kt in range(KT_H):
            t = T(pp, [128, EMB], SEQ_DT, f"wtrans{kt}")
            dma(t[:], wtrans[kt * 128:(kt + 1) * 128, :])
            wtrans_t.append(t)

        ps_seq = [T(pps, [128, EMB], FP32, "ps") for _ in range(8)]
        for kt in range(KT_H):
            xt_t = T(pst, [128, C], SEQ_DT, "xt_stream", bufs=2)
            dma(xt_t[:], xt[kt * 128:(kt + 1) * 128, :])
            for mt in range(8):
                nc.tensor.matmul(
                    ps_seq[mt][:], xt_t[:, mt * 128:(mt + 1) * 128], wtrans_t[kt][:],
                    start=(kt == 0), stop=(kt == KT_H - 1))
        seq_t = []
        for mt in range(8):
            t = T(pp, [128, EMB], SEQ_DT, f"seq{mt}")
            nc.vector.tensor_add(t[:], ps_seq[mt][:], btrans_bc[:])
            seq_t.append(t)

        # ---- S2: mention embeddings + entity logsumexp nodes ----
        ps_memb = T(pps, [E * M, EMB], FP32, "ps")
        for kt in range(KT_H):
            xg_t = T(pst, [128, E * M], SEQ_DT, "xg_stream", bufs=3)
            dma(xg_t[:], xg[kt * 128:(kt + 1) * 128, :])
            nc.tensor.matmul(ps_memb[:], xg_t[:], wtrans_t[kt][:],
                             start=(kt == 0), stop=(kt == KT_H - 1))
        memb_t = T(pp, [E * M, EMB], FP32, "memb")
        nc.vector.tensor_add(memb_t[:], ps_memb[:], btrans_bc[0:E * M, :])
        nc.vector.tensor_copy(nodes_m[:, 0:EMB], memb_t[:])
        ememb_t = T(pp, [E * M, EMB], FP32, "ememb")
        nc.scalar.activation(ememb_t[:], memb_t[:], ACT.Exp)
        g3_t = T(pp, [E * M, E], FP32, "g3")
        dma(g3_t[:], g3)
        ps_ent = T(pps, [E, EMB], FP32, "ps")
        nc.tensor.matmul(ps_ent[:], g3_t[:], ememb_t[:], start=True, stop=True)
        nc.scalar.activation(nodes_e[:, 0:EMB], ps_ent[:], ACT.Ln)

        # ---- S3: link nodes ----
        # a[s] = mean over (h,i) of the 16x16 link attention block, s=(l,j)
        aT_t, aTb_t, xspan_t, gspan_t = [], [], [], []
        for i, (off, sz) in enumerate(_ts(SPAN_TILES)):
            al = T(pst, [sz, NH * LS], FP32, "attl_stream", bufs=2)
            dma(al[:], attl[off:off + sz, :])
            a = T(pp, [sz, 1], FP32, f"aT{i}")
            nc.vector.tensor_reduce(a[:], al[:], mybir.AxisListType.X,
                                    mybir.AluOpType.add)
            nc.vector.tensor_scalar_mul(a[:], a[:], 1.0 / (NH * LS))
            aT_t.append(a)
            ab = T(pp, [sz, 1], SEQ_DT, f"aTb{i}")
            nc.vector.tensor_copy(ab[:], a[:])
            aTb_t.append(ab)
            gs = T(pp, [sz, L], SEQ_DT, f"gspan{i}")
            dma(gs[:], gspan[off:off + sz, :])
            gspan_t.append(gs)
            xs = T(pp, [sz, H], SEQ_DT, f"xspan{i}")
            dma(xs[:], xspan[off:off + sz, :])
            xspan_t.append(xs)
        # asum[l] = sum_j a_l[j] (for the bias term); uses unscaled-by-X a
        ps_as = T(pps, [L, 1], FP32, "ps")
        for kt in range(4):
            nc.tensor.matmul(ps_as[:], gspan_t[kt][:], aTb_t[kt][:],
                             start=(kt == 0), stop=(kt == 3))
        asum_t = T(pp, [L, 1], FP32, "asum")
        nc.vector.tensor_copy(asum_t[:], ps_as[:])
        # scale xspan rows by a in place, then project through gspan
        for kt in range(4):
            nc.vector.tensor_scalar_mul(xspan_t[kt][:], xspan_t[kt][:],
                                        aT_t[kt][:])
        # linkctxT [768, 30]
        lct_t = []
        for mt in range(KT_H):
            ps = T(pps, [128, L], FP32, "ps")
            for kt in range(4):
                nc.tensor.matmul(ps[:], xspan_t[kt][:, mt * 128:(mt + 1) * 128],
                                 gspan_t[kt][:], start=(kt == 0), stop=(kt == 3))
            t = T(pp, [128, L], SEQ_DT, f"lct{mt}")
            nc.vector.tensor_copy(t[:], ps[:])
            lct_t.append(t)
        bterm_t = T(pp, [L, EMB], FP32, "bterm")
        nc.vector.tensor_scalar_mul(bterm_t[:], btrans_bc[0:L, :], asum_t[:])
        ps_link = T(pps, [L, EMB], FP32, "ps")
        for kt in range(KT_H):
            nc.tensor.matmul(ps_link[:], lct_t[kt][:], wtrans_t[kt][:],
                             start=(kt == 0), stop=(kt == KT_H - 1))
        nc.vector.tensor_add(nodes_l[:, 0:EMB], ps_link[:], bterm_t[:])

        # ---- S4: ea (entity attention) + e_ctx ----
        ps_ea = [T(pps, [E, 512], FP32, "ps") for _ in range(2)]
        n_attm = len(ATTM_TILES)
        for i, (off, sz) in enumerate(_ts(ATTM_TILES)):
            at = T(pst, [sz, C], GRAPH_DT, "attm_stream", bufs=2)
            dma(at[:], attm[off:off + sz, :])
            gt = T(pst, [sz, E], GRAPH_DT, "gmat_stream", bufs=3)
            dma(gt[:], gmat[off:off + sz, :])
            for half in range(2):
                nc.tensor.matmul(ps_ea[half][:], gt[:],
                                 at[:, half * 512:(half + 1) * 512],
                                 start=(i == 0), stop=(i == n_attm - 1))
        ea_t = T(pp, [E, C], FP32, "ea")
        for half in range(2):
            nc.vector.tensor_copy(ea_t[:, half * 512:(half + 1) * 512],
                                  ps_ea[half][:])
        rsum_t = T(pp, [E, 1], FP32, "rsum")
        nc.vector.tensor_reduce(rsum_t[:], ea_t[:], mybir.AxisListType.X,
                                mybir.AluOpType.add)
        nc.vector.tensor_scalar_add(rsum_t[:], rsum_t[:], 1e-5)
        recip_t = T(pp, [E, 1], FP32, "recip")
        nc.vector.reciprocal(recip_t[:], rsum_t[:])
        ean_t = T(pp, [E, C], FP32, "ean")
        nc.vector.tensor_scalar_mul(ean_t[:], ea_t[:], recip_t[:])
        # eaNT via PE transpose, then e_ctx [22, 512]
        ps_ectx = T(pps, [E, EMB], FP32, "ps")
        for kt in range(8):
            pst_ea = T(pps, [128, E], FP32, "ps")
            nc.tensor.transpose(pst_ea[:], ean_t[:, kt * 128:(kt + 1) * 128],
                                ident_t[0:E, 0:E])
            eaT = T(pst, [128, E], SEQ_DT, "eaT_stream", bufs=3)
            nc.vector.tensor_copy(eaT[:], pst_ea[:])
            nc.tensor.matmul(ps_ectx[:], eaT[:], seq_t[kt][:],
                             start=(kt == 0), stop=(kt == 7))
        ectx_t = T(pp, [E, EMB], FP32, "ectx")
        nc.vector.tensor_copy(ectx_t[:], ps_ectx[:])

        def conv_pass(ps_c, in_tiles, w_dram, n_ic_t, n_oc_t, ocs,
                      first, last):
            """One accumulation pass of a 5x5 SAME conv on 22x22."""
            n_acc = 25 * n_ic_t
            a = 0
            for tap in range(25):
                di, dj = divmod(tap, 5)
                for kt in range(n_ic_t):
                    w = T(pst, [128, ocs], CONV_DT, "wconv_stream", bufs=6)
                    dma(w[:], w_dram[(tap * n_ic_t + kt) * 128:
                                     (tap * n_ic_t + kt + 1) * 128, :])
                    rhs = in_tiles[kt][:].rearrange(
                        "p (a b) -> p a b", a=PW, b=PW)[:, di:di + S, dj:dj + S]
                    for mt in range(n_oc_t):
                        nc.tensor.matmul(ps_c[mt][:], w[:, mt * 128:(mt + 1) * 128],
                                         rhs, start=(first and a == 0),
                                         stop=(last and a == n_acc - 1))
                    a += 1

        def conv(in_tiles, w_dram, n_ic_t, n_oc_t, ocs, bias_tiles, out_cb):
            ps_c = [T(pps, [128, SP], FP32, "ps") for _ in range(n_oc_t)]
            conv_pass(ps_c, in_tiles, w_dram, n_ic_t, n_oc_t, ocs, True, True)
            for mt in range(n_oc_t):
                out_cb(mt, ps_c[mt])

        # ---- S5: RGCN ----
        # nodesT [532, 118] built from the three node-group tiles
        NODE_GROUPS = [(0, E), (E, E * M), (E + E * M, L)]
        node_tiles = [nodes_e, nodes_m, nodes_l]
        # bf16 copies of the node features for the rgcn matmuls
        nodes_bf = []
        for gi, (goff, gsz) in enumerate(NODE_GROUPS):
            t = T(pp, [gsz, NF], GRAPH_DT, f"nodes_bf{gi}")
            nc.vector.tensor_copy(t[:], node_tiles[gi][:])
            nodes_bf.append(t)
        nodesT_t = []
        for i, (off, sz) in enumerate(_ts(NF_TILES)):
            t = T(pp, [sz, NN], GRAPH_DT, f"nodesT{i}")
            for gi, (goff, gsz) in enumerate(NODE_GROUPS):
                ps = T(pps, [sz, 128], FP32, "ps")
                nc.tensor.transpose(ps[0:sz, 0:gsz],
                                    node_tiles[gi][:, off:off + sz],
                                    ident_t[0:gsz, 0:gsz])
                nc.vector.tensor_copy(t[:, goff:goff + gsz], ps[0:sz, 0:gsz])
            nodesT_t.append(t)
        # adjacency normalize; adjn split into the three node row-groups
        adjt_t = T(pp, [NN, 4 * NN], FP32, "adjt")
        dma(adjt_t[:], adjt)
        ps_rs = T(pps, [1, 4 * NN], FP32, "ps")
        nc.tensor.matmul(ps_rs[:], ones_t[0:NN, 0:1], adjt_t[:],
                         start=True, stop=True)
        rs_t = T(pp, [1, 4 * NN], FP32, "rs")
        nc.vector.tensor_scalar_add(rs_t[:], ps_rs[:], 1e-5)
        rcp_t = T(pp, [1, 4 * NN], FP32, "rcp")
        nc.vector.reciprocal(rcp_t[:], rs_t[:])
        rsbc_t = T(pp, [128, 4 * NN], FP32, "rsbc")
        nc.gpsimd.partition_broadcast(rsbc_t[:], rcp_t[:])
        adjn_t = []
        for gi, (goff, gsz) in enumerate(NODE_GROUPS):
            tf = T(pst, [gsz, 4 * NN], FP32, "adjn_f32", bufs=3)
            dma(tf[:], adjt[goff:goff + gsz, :])
            t = T(pp, [gsz, 4 * NN], GRAPH_DT, f"adjn{gi}")
            nc.vector.tensor_mul(t[:], tf[:], rsbc_t[0:gsz, :])
            adjn_t.append(t)

        ps_gcn = [T(pps, [128, NN], FP32, "ps") for _ in range(4)]
        n_terms = 5 * 5  # (4 rel + self) x 5 k-tiles
        term = 0
        for r in range(5):
            # r<4: relation r via msgT; r==4: self term via nodesT
            if r < 4:
                msgT_t = []
                for i, (off, sz) in enumerate(_ts(NF_TILES)):
                    ps = T(pps, [sz, NN], FP32, "ps")
                    for gi, (goff, gsz) in enumerate(NODE_GROUPS):
                        nc.tensor.matmul(ps[:], nodes_bf[gi][:, off:off + sz],
                                         adjn_t[gi][:, r * NN:(r + 1) * NN],
                                         start=(gi == 0), stop=(gi == 2))
                    t = T(pst, [sz, NN], GRAPH_DT, f"msgT_stream{i}", bufs=2)
                    nc.vector.tensor_copy(t[:], ps[:])
                    msgT_t.append(t)
                rhs_t = msgT_t
            else:
                rhs_t = nodesT_t
            for i, (off, sz) in enumerate(_ts(NF_TILES)):
                w = T(pst, [sz, EMB], GRAPH_DT, "wg_stream", bufs=3)
                if r < 4:
                    dma(w[:], wrel[r * NF + off:r * NF + off + sz, :])
                else:
                    dma(w[:], wself[off:off + sz, :])
                for mt in range(4):
                    nc.tensor.matmul(ps_gcn[mt][:], w[:, mt * 128:(mt + 1) * 128],
                                     rhs_t[i][:], start=(term == 0),
                                     stop=(term == n_terms - 1))
                term += 1
        # [512,1] needs 4 partition tiles
        brgcn_tiles = []
        for mt in range(4):
            t = T(pp, [128, 1], FP32, f"brgcn{mt}")
            dma(t[:], brgcn[mt * 128:(mt + 1) * 128, :])
            brgcn_tiles.append(t)
        gcnT_t = []
        for mt in range(4):
            t = T(pp, [128, NN], FP32, f"gcnT{mt}")
            nc.scalar.activation(t[:], ps_gcn[mt][:], ACT.Relu,
                                 bias=brgcn_tiles[mt][:, 0:1])
            gcnT_t.append(t)
        # ent natural [22, 512]; entT view = gcnT[:, 0:22]
        ent_t = T(pp, [E, EMB], FP32, "ent")
        for mt in range(4):
            ps = T(pps, [E, 128], FP32, "ps")
            nc.tensor.transpose(ps[:], gcnT_t[mt][:, 0:E], ident_t[:, :])
            nc.vector.tensor_copy(ent_t[:, mt * 128:(mt + 1) * 128], ps[:])
        # ectxT tiles [128, 22] x4
        ectxT_t = []
        for mt in range(4):
            ps = T(pps, [128, E], FP32, "ps")
            nc.tensor.transpose(ps[:], ectx_t[:, mt * 128:(mt + 1) * 128],
                                ident_t[0:E, 0:E])
            t = T(pp, [128, E], FP32, f"ectxT{mt}")
            nc.vector.tensor_copy(t[:], ps[:])
            ectxT_t.append(t)

        # ---- S6: relation map x + conv stack ----
        xpad_t = []
        for mt in range(4):
            xp = T(pp, [128, SPP], CONV_DT, f"xpad{mt}")
            nc.vector.memset(xp[:], 0.0)
            entT_v = gcnT_t[mt][:, 0:E]
            t1 = T(pp, [128, SP], FP32, "xtmp1")
            nc.vector.tensor_mul(
                t1[:].rearrange("p (a b) -> p a b", a=S, b=S),
                entT_v.unsqueeze(2).to_broadcast((128, S, S)),
                entT_v.unsqueeze(1).to_broadcast((128, S, S)))
            t2 = T(pp, [128, SP], FP32, "xtmp2")
            nc.vector.tensor_mul(
                t2[:].rearrange("p (a b) -> p a b", a=S, b=S),
                ectxT_t[mt][:].unsqueeze(2).to_broadcast((128, S, S)),
                ectxT_t[mt][:].unsqueeze(1).to_broadcast((128, S, S)))
            inner = xp[:].rearrange("p (a b) -> p a b", a=PW, b=PW)[:, 2:2 + S, 2:2 + S]
            nc.vector.tensor_add(inner, t1[:], t2[:])
            xpad_t.append(xp)

        # conv1: 512 -> 256, output into padded tiles for conv2
        pad1_t = []
        for mt in range(2):
            t = T(pp, [128, SPP], CONV_DT, f"pad1_{mt}")
            nc.vector.memset(t[:], 0.0)
            pad1_t.append(t)
        b1_tiles = []
        for mt in range(2):
            t = T(pp, [128, 1], FP32, f"b1_{mt}")
            dma(t[:], b1[mt * 128:(mt + 1) * 128, :])
            b1_tiles.append(t)

        def c1_out(mt, ps):
            inner = pad1_t[mt][:].rearrange("p (a b) -> p a b", a=PW, b=PW)[
                :, 2:2 + S, 2:2 + S]
            nc.scalar.activation(inner, ps[:].rearrange("p (a b) -> p a b", a=S, b=S),
                                 ACT.Relu, bias=b1_tiles[mt][:, 0:1])

        conv(xpad_t, w1t, 4, 2, IC, b1_tiles, c1_out)

        pad2_t = []
        for mt in range(2):
            t = T(pp, [128, SPP], CONV_DT, f"pad2_{mt}")
            nc.vector.memset(t[:], 0.0)
            pad2_t.append(t)
        b2_tiles = []
        for mt in range(2):
            t = T(pp, [128, 1], FP32, f"b2_{mt}")
            dma(t[:], b2[mt * 128:(mt + 1) * 128, :])
            b2_tiles.append(t)

        def c2_out(mt, ps):
            inner = pad2_t[mt][:].rearrange("p (a b) -> p a b", a=PW, b=PW)[
                :, 2:2 + S, 2:2 + S]
            nc.scalar.activation(inner, ps[:].rearrange("p (a b) -> p a b", a=S, b=S),
                                 ACT.Relu, bias=b2_tiles[mt][:, 0:1])

        conv(pad1_t, w2t, 2, 2, IC, b2_tiles, c2_out)

        x3_t = []
        b3_tiles = []
        for mt in range(4):
            t = T(pp, [128, 1], FP32, f"b3_{mt}")
            dma(t[:], b3[mt * 128:(mt + 1) * 128, :])
            b3_tiles.append(t)
        for mt in range(4):
            t = T(pp, [128, SP], PAIR_DT, f"x3_{mt}")
            x3_t.append(t)

        def c3_out(mt, ps):
            nc.scalar.activation(x3_t[mt][:], ps[:], ACT.Relu,
                                 bias=b3_tiles[mt][:, 0:1])

        conv(pad2_t, w3t, 2, 4, EMB, b3_tiles, c3_out)

        # ---- S7: pair features + classifier ----
        # x3T [484, 512]
        x3T_t = []
        for i, (off, sz) in enumerate(_ts(SP_TILES)):
            t = T(pp, [sz, EMB], PAIR_DT, f"x3T{i}")
            x3T_t.append(t)
        for i, (off, sz) in enumerate(_ts(SP_TILES)):
            for src in range(4):
                ps = T(pps, [sz, 64], FP32, "ps")
                psb = ps[:].bitcast(PAIR_DT)
                nc.tensor.transpose(psb, x3_t[src][:, off:off + sz],
                                    identp_t[:, :])
                nc.vector.tensor_copy(x3T_t[i][:, src * 128:(src + 1) * 128], psb)

        sh_t = T(pp, [E, PH], FP32, "sh")
        dma(sh_t[:], sh)
        st_t = T(pp, [E, PH], FP32, "st")
        dma(st_t[:], st)
        sm_t = []
        for i, (off, sz) in enumerate(_ts(SP_TILES)):
            t = T(pp, [sz, PH], PAIR_DT, f"sm{i}")
            dma(t[:], sm[off:off + sz, :])
            sm_t.append(t)

        featT = [None] * 16
        for mt in range(4):
            ps = T(pps, [128, PH], FP32, "ps")
            nc.tensor.matmul(ps[:], ent_t[:, mt * 128:(mt + 1) * 128], sh_t[:],
                             start=True, stop=True)
            t = T(pp, [128, PH], PAIR_DT, f"featT{mt}")
            nc.vector.tensor_copy(t[:], ps[:])
            featT[mt] = t
        for mt in range(4):
            ps = T(pps, [128, PH], FP32, "ps")
            nc.tensor.matmul(ps[:], ent_t[:, mt * 128:(mt + 1) * 128], st_t[:],
                             start=True, stop=True)
            t = T(pp, [128, PH], PAIR_DT, f"featT{4 + mt}")
            nc.vector.tensor_copy(t[:], ps[:])
            featT[4 + mt] = t
        for mt in range(4):
            ps = T(pps, [128, PH], FP32, "ps")
            for i, (off, sz) in enumerate(_ts(SP_TILES)):
                nc.tensor.matmul(ps[:], x3T_t[i][:, mt * 128:(mt + 1) * 128],
                                 sm_t[i][:], start=(i == 0), stop=(i == 3))
            t = T(pp, [128, PH], PAIR_DT, f"featT{8 + mt}")
            nc.vector.tensor_copy(t[:], ps[:])
            featT[8 + mt] = t
        for mt in range(4):
            t = T(pp, [128, PH], PAIR_DT, f"featT{12 + mt}")
            nc.vector.tensor_mul(t[:], featT[mt][:], featT[4 + mt][:])
            featT[12 + mt] = t

        bht_tiles = []
        for mt in range(8):
            t = T(pp, [128, 1], FP32, f"bht{mt}")
            dma(t[:], bht[mt * 128:(mt + 1) * 128, :])
            bht_tiles.append(t)
        ps_ht = [T(pps, [128, PH], FP32, "ps") for _ in range(8)]
        for kt in range(16):
            w = T(pst, [128, 2 * EMB], PAIR_DT, "wht_stream", bufs=4)
            dma(w[:], wht[kt * 128:(kt + 1) * 128, :])
            for mt in range(8):
                nc.tensor.matmul(ps_ht[mt][:], w[:, mt * 128:(mt + 1) * 128],
                                 featT[kt][:], start=(kt == 0), stop=(kt == 15))
        htT_t = []
        for mt in range(8):
            t = T(pp, [128, PH], PAIR_DT, f"htT{mt}")
            nc.scalar.activation(t[:], ps_ht[mt][:], ACT.Tanh,
                                 bias=bht_tiles[mt][:, 0:1])
            htT_t.append(t)

        ps_out = T(pps, [97, PH], FP32, "ps")
        for kt in range(8):
            w = T(pst, [128, 97], PAIR_DT, "wbil_stream", bufs=3)
            dma(w[:], wbil[kt * 128:(kt + 1) * 128, :])
            nc.tensor.matmul(ps_out[:], w[:], htT_t[kt][:],
                             start=(kt == 0), stop=(kt == 7))
        bbil_t = T(pp, [97, 1], FP32, "bbil")
        dma(bbil_t[:], bbil)
        out_t = T(pp, [97, PH], FP32, "out")
        nc.vector.tensor_scalar_add(out_t[:], ps_out[:], bbil_t[:, 0:1])
        dma(outt, out_t[:])

    nc.compile()
    return nc


_PROG = None


def _get_prog():
    global _PROG
    if _PROG is None:
        _PROG = build_program()
    return _PROG


def _np(dt):
    return _NPDT[dt]


def _shared_inputs(inputs):
    f32 = np.float32
    sh = {}
    sh["wtrans"] = np.ascontiguousarray(inputs["W_trans"], _np(SEQ_DT))
    sh["btrans"] = np.ascontiguousarray(inputs["b_trans"], f32).reshape(1, EMB)
    sh["gmat"] = np.kron(np.eye(E, dtype=f32),
                         np.ones((M * NH, 1), f32) / (M * NH)).astype(_np(GRAPH_DT))
    sh["g3"] = np.kron(np.eye(E, dtype=f32), np.ones((M, 1), f32))
    sh["gspan"] = np.kron(np.eye(L, dtype=f32), np.ones((LS, 1), f32)).astype(_np(SEQ_DT))
    sh["ones"] = np.ones((128, 1), f32)
    sh["ident"] = np.eye(128, dtype=f32)
    sh["identp"] = np.eye(128, dtype=_np(PAIR_DT))
    sh["wrel"] = np.ascontiguousarray(inputs["W_rel"], f32).reshape(4 * NF, EMB).astype(_np(GRAPH_DT))
    sh["wself"] = np.ascontiguousarray(inputs["W_self"], f32).astype(_np(GRAPH_DT))
    sh["brgcn"] = np.ascontiguousarray(inputs["b_rgcn"], f32).reshape(EMB, 1)
    sh["w1t"] = np.ascontiguousarray(
        np.asarray(inputs["conv1_w"], f32).transpose(2, 3, 1, 0).reshape(25 * EMB, IC),
        _np(CONV_DT))
    sh["b1"] = np.ascontiguousarray(inputs["conv1_b"], f32).reshape(IC, 1)
    sh["w2t"] = np.ascontiguousarray(
        np.asarray(inputs["conv2_w"], f32).transpose(2, 3, 1, 0).reshape(25 * IC, IC),
        _np(CONV_DT))
    sh["b2"] = np.ascontiguousarray(inputs["conv2_b"], f32).reshape(IC, 1)
    sh["w3t"] = np.ascontiguousarray(
        np.asarray(inputs["conv3_w"], f32).transpose(2, 3, 1, 0).reshape(25 * IC, EMB),
        _np(CONV_DT))
    sh["b3"] = np.ascontiguousarray(inputs["conv3_b"], f32).reshape(EMB, 1)
    sh["wht"] = np.ascontiguousarray(inputs["ht_W"], _np(PAIR_DT))
    sh["bht"] = np.ascontiguousarray(inputs["ht_b"], f32).reshape(2 * EMB, 1)
    sh["wbil"] = np.ascontiguousarray(inputs["bil_W"], _np(PAIR_DT))
    sh["bbil"] = np.ascontiguousarray(inputs["bil_b"], f32).reshape(97, 1)
    return sh


def _core_inputs(inputs, shared, b, hh):
    f32 = np.float32
    X = np.asarray(inputs["sequence_output"][b], f32)
    att = np.asarray(inputs["attention"][b], f32)
    adj = np.asarray(inputs["adjacency"][b], f32)
    mf = np.asarray(inputs["mention_idx"][b]).reshape(-1).astype(np.int64)
    ls = np.asarray(inputs["link_start"][b]).reshape(-1).astype(np.int64)
    ntypes = np.asarray(inputs["node_types"][b]).astype(np.int64)
    hts = np.asarray(inputs["hts"][b]).astype(np.int64)

    m = dict(shared)
    m["xt"] = np.ascontiguousarray(X.T, _np(SEQ_DT))
    m["xg"] = np.ascontiguousarray(X[mf].T, _np(SEQ_DT))
    pos = ls[:, None] + np.arange(LS)
    m["xspan"] = np.ascontiguousarray(X[pos.reshape(-1)]).astype(_np(SEQ_DT))
    rows = att[:, mf, :]
    m["attm"] = np.ascontiguousarray(
        rows.transpose(1, 0, 2).reshape(ATTM_ROWS, C)).astype(_np(GRAPH_DT))
    attl = np.empty((SPAN_ROWS, NH * LS), f32)
    for l in range(L):
        blk = att[:, pos[l], :][:, :, pos[l]]           # [12, 16i, 16j]
        attl[l * LS:(l + 1) * LS, :] = blk.transpose(2, 0, 1).reshape(LS, NH * LS)
    m["attl"] = attl
    m["adjt"] = np.ascontiguousarray(
        np.concatenate([adj[r].T for r in range(4)], axis=1), f32)
    m["typ"] = np.ascontiguousarray(
        np.asarray(inputs["type_embed"], f32)[ntypes], f32)
    pr = hts[hh * PH:(hh + 1) * PH]
    shm = np.zeros((E, PH), f32)
    shm[pr[:, 0], np.arange(PH)] = 1.0
    stm = np.zeros((E, PH), f32)
    stm[pr[:, 1], np.arange(PH)] = 1.0
    smm = np.zeros((SP, PH), f32)
    smm[pr[:, 0] * E + pr[:, 1], np.arange(PH)] = 1.0
    m["sh"] = shm
    m["st"] = stm
    m["sm"] = np.ascontiguousarray(smm, _np(PAIR_DT))
    return m


def kernel(**inputs):
    nc = _get_prog()
    shared = _shared_inputs(inputs)
    in_maps = []
    for b in range(B):
        for hh in range(2):
            in_maps.append(_core_inputs(inputs, shared, b, hh))
    res = run_bass_kernel_spmd(nc, in_maps, core_ids=list(range(8)))
    out = np.empty((B, P, 97), np.float32)
    for b in range(B):
        for hh in range(2):
            out[b, hh * PH:(hh + 1) * PH, :] = np.asarray(
                res.results[2 * b + hh]["outt"], np.float32).T
    return out
